# revision 3
# baseline (speedup 1.0000x reference)
"""DGCN (EdgeConv x2 + DynamicEdgeConv + readout MLP) on 8 TRN2 NeuronCores.

Sharding: graph-level data parallel. 64 graphs -> 8 cores x 8 graphs.
Within a core, graphs are processed as 2 "packs" of 4 graphs (4 x 32ch = 128
partitions). All activations live transposed (channels/features on the
partition axis, nodes/edges on the free axis) so that:
  - projections/Gram matrices are plain matmuls over the feature axis,
  - EdgeConv neighbor gathers are single GPSIMD ap_gather ops along the
    free axis (per-16-partition-group index lists),
  - per-edge MLPs are block-diagonal 128-contraction matmuls,
  - BatchNorm affine+LeakyReLU collapse into one ScalarE activation.
BatchNorm statistics are global over all 17152 nodes -> one tiny AllReduce;
the readout MLP (BN over the 64-graph batch) runs replicated on every core
after an AllGather of the 332-dim per-graph feature vectors.
"""

import os
import sys

sys.path.insert(0, "/opt/trn_rl_repo")

import numpy as np

B = 64
ROI = 268
F = 268
C = 32
K = 32
DEG = 32
NCORES = 8
GPC = B // NCORES          # graphs per core = 8
NLOC = GPC * ROI           # nodes per core = 2144
NTOT = B * ROI             # 17152
EG = ROI * DEG             # edges per graph = 8576
PACKS = 2                  # 4-graph packs per core
SLOPE = 0.33
EPS = 1e-5

_cache = {}


def _fp32(a):
    return np.ascontiguousarray(a, dtype=np.float32)


def _build_program():
    import concourse.bacc as bacc
    import concourse.tile as tile
    import concourse.mybir as mybir
    from concourse import bass

    dt = mybir.dt
    f32r = dt.float32r
    AF = mybir.ActivationFunctionType
    OP = mybir.AluOpType
    AX = mybir.AxisListType

    taps = False
    single = bool(int(os.environ.get("KERNEL_SINGLE", "0")))

    nc = bacc.Bacc("TRN2", target_bir_lowering=False, debug=False,
                   num_devices=1 if single else NCORES)

    # ---- DRAM I/O -------------------------------------------------------
    d_xT = nc.dram_tensor("xT", [F, NLOC], dt.float32, kind="ExternalInput")
    d_idx1 = nc.dram_tensor("idx1", [2 * 128, EG // 16], dt.int16, kind="ExternalInput")
    d_ident = nc.dram_tensor("ident", [128, 128], dt.float32, kind="ExternalInput")
    d_wproj = nc.dram_tensor("wproj", [F, 128], dt.float32, kind="ExternalInput")
    d_wproj3 = nc.dram_tensor("wproj3", [F, 2], dt.float32, kind="ExternalInput")
    d_wbd1 = nc.dram_tensor("wbd1", [128, 128], dt.bfloat16, kind="ExternalInput")
    d_wbd2 = nc.dram_tensor("wbd2", [128, 128], dt.bfloat16, kind="ExternalInput")
    d_bpack = nc.dram_tensor("bpack", [128, 2], dt.float32, kind="ExternalInput")
    d_fold = nc.dram_tensor("foldM", [128, 33], dt.float32, kind="ExternalInput")
    d_sel = nc.dram_tensor("selM", [33, 256], dt.float32, kind="ExternalInput")
    d_gbe = nc.dram_tensor("gbe", [33, 4], dt.float32, kind="ExternalInput")
    d_mask3 = nc.dram_tensor("mask3", [128, 24 * ROI], dt.bfloat16,
                             kind="ExternalInput")
    d_wl1 = nc.dram_tensor("wl1", [332, 256], dt.float32, kind="ExternalInput")
    d_wl2 = nc.dram_tensor("wl2", [256, 128], dt.float32, kind="ExternalInput")
    d_wl3 = nc.dram_tensor("wl3", [128, 1], dt.float32, kind="ExternalInput")
    d_gbe4 = nc.dram_tensor("gbe4", [128, 4], dt.float32, kind="ExternalInput")
    d_gbe5 = nc.dram_tensor("gbe5", [128, 2], dt.float32, kind="ExternalInput")
    d_bl3 = nc.dram_tensor("bl3", [1, 1], dt.float32, kind="ExternalInput")
    d_out = nc.dram_tensor("out", [1, B], dt.float32, kind="ExternalOutput")

    d_taps = {}
    if taps:
        for nm, shp in [("tap_a1T0", [128, ROI]), ("tap_a1T1", [128, ROI]),
                        ("tap_a2T0", [128, ROI]), ("tap_a2T1", [128, ROI]),
                        ("tap_a3row", [128, ROI]), ("tap_stats", [33, 6]),
                        ("tap_aff1", [128, 4]), ("tap_pool", [128, 4]),
                        ("tap_za", [128, 64]), ("tap_zb", [128, 64]), ("tap_zc", [76, 64]), ("tap_wr0", [16, EG // 16]),
                        ("tap_key0", [128, ROI])]:
            d_taps[nm] = nc.dram_tensor(nm, shp, dt.float32, kind="ExternalOutput")

    FCH = [(0, 128), (128, 128), (256, 12)]      # feature-axis chunks
    ECH = [(i * 512, 512) for i in range(16)] + [(16 * 512, EG - 16 * 512)]

    with tile.TileContext(nc) as tc:
        with tc.tile_pool(name="const", bufs=1) as wpool, \
             tc.tile_pool(name="persist", bufs=1) as ppool, \
             tc.tile_pool(name="xt", bufs=2) as xpool, \
             tc.tile_pool(name="edge", bufs=2) as epool, \
             tc.tile_pool(name="scratch", bufs=2) as spool, \
             tc.tile_pool(name="psA", bufs=2, space="PSUM") as psA, \
             tc.tile_pool(name="psB", bufs=2, space="PSUM") as psB, \
             tc.tile_pool(name="psM", bufs=3, space="PSUM") as psM, \
             tc.tile_pool(name="dram", bufs=1, space="DRAM") as dpool:

            # ---- constants to SBUF -------------------------------------
            def load(name, shape, dtype, src):
                t = wpool.tile(shape, dtype, tag=name)
                nc.sync.dma_start(t[:], src)
                return t

            ident = load("ident", [128, 128], dt.float32, d_ident.ap())
            wproj = [load(f"wproj{i}", [sz, 128], dt.float32,
                          d_wproj.ap()[o:o + sz, :]) for i, (o, sz) in enumerate(FCH)]
            wproj3 = [load(f"wproj3{i}", [sz, 2], dt.float32,
                           d_wproj3.ap()[o:o + sz, :]) for i, (o, sz) in enumerate(FCH)]
            wbd = [load("wbd1", [128, 128], dt.bfloat16, d_wbd1.ap()),
                   load("wbd2", [128, 128], dt.bfloat16, d_wbd2.ap())]
            bpack = load("bpack", [128, 2], dt.float32, d_bpack.ap())
            foldM = load("foldM", [128, 33], dt.float32, d_fold.ap())
            selM = load("selM", [33, 256], dt.float32, d_sel.ap())
            gbe = load("gbe", [33, 4], dt.float32, d_gbe.ap())
            mask3 = load("mask3", [128, 24 * ROI], dt.bfloat16, d_mask3.ap())
            MCH = [(0, 128), (128, 128), (256, 76)]   # 332 rows of wl1 / zT
            wl1 = [load(f"wl1_{i}", [sz, 256], dt.float32,
                        d_wl1.ap()[o:o + sz, :]) for i, (o, sz) in enumerate(MCH)]
            wl2 = [load(f"wl2_{i}", [128, 128], dt.float32,
                        d_wl2.ap()[128 * i:128 * i + 128, :]) for i in range(2)]
            wl3 = load("wl3", [128, 1], dt.float32, d_wl3.ap())
            gbe4 = load("gbe4", [128, 4], dt.float32, d_gbe4.ap())
            gbe5 = load("gbe5", [128, 2], dt.float32, d_gbe5.ap())
            bl3 = load("bl3", [1, 1], dt.float32, d_bl3.ap())
            idx1sb = [load(f"idx1_{t}", [128, EG // 16], dt.int16,
                           d_idx1.ap()[128 * t:128 * t + 128, :]) for t in range(2)]

            ones_col = wpool.tile([128, 1], dt.float32, tag="ones_col", name="ones_col")
            nc.vector.memset(ones_col[:], 1.0)
            ones_row = wpool.tile([1, 128], dt.float32, tag="ones_row", name="ones_row")
            nc.vector.memset(ones_row[:], 1.0)

            # ---- persistent per-core tensors ---------------------------
            Vp = [[ppool.tile([128, ROI], dt.float32, tag=f"V{cv}p{pk}", name=f"V{cv}p{pk}")
                   for pk in range(PACKS)] for cv in range(2)]
            Up = [[ppool.tile([128, ROI], dt.float32, tag=f"U{cv}p{pk}", name=f"U{cv}p{pk}")
                   for pk in range(PACKS)] for cv in range(2)]
            aT = [[ppool.tile([128, ROI], dt.float32, tag=f"a{cv}p{pk}", name=f"a{cv}p{pk}")
                   for pk in range(PACKS)] for cv in range(2)]
            u3row = ppool.tile([1, NLOC], dt.float32, tag="u3row", name="u3row")
            v3row = ppool.tile([1, NLOC], dt.float32, tag="v3row", name="v3row")
            A3all = ppool.tile([128, 24], dt.float32, tag="A3all", name="A3all")
            t3scr = ppool.tile([128, ROI], dt.float32, tag="t3scr", name="t3scr")
            packR = ppool.tile([96, ROI], dt.float32, tag="packR", name="packR")
            wrapped = [ppool.tile([16, EG // 16], dt.int16, tag=f"wr{g}", name=f"wr{g}")
                       for g in range(GPC)]
            stats6 = ppool.tile([128, 10], dt.float32, tag="stats6", name="stats6")
            nc.vector.memset(A3all[:], 0.0)
            sq_scratch = ppool.tile([128, ROI], dt.float32, tag="sq_scratch", name="sq_scratch")

            # ================================================================
            # Stage 1: per graph-pair: load xT, squares, projections, d2 + topk
            # ================================================================
            def topk32(keyS, csz, gl, ic):
                """keyS [csz<=128, ROI] f32 SBUF (destroyed). Writes wrapped[gl]
                columns for i-chunk ic (ic in 0,1) or returns idxf for packR."""
                idxu = spool.tile([128, K], dt.uint32, tag="idxu", name="idxu")
                for r in range(4):
                    m8 = spool.tile([128, 8], dt.float32, tag=f"m8_{r % 2}", name=f"m8_{r % 2}")
                    nc.vector.max(m8[:csz, :], keyS[:csz, :])
                    nc.vector.max_index(idxu[:csz, 8 * r:8 * r + 8], m8[:csz, :],
                                        keyS[:csz, :])
                    if r < 3:
                        nc.vector.match_replace(keyS[:csz, :], m8[:csz, :],
                                                keyS[:csz, :], -1e30)
                idxf = spool.tile([128, K], dt.float32, tag="idxf", name="idxf")
                nc.vector.tensor_copy(idxf[:csz, :], idxu[:csz, :])
                return idxf

            def idx_to_wrapped(idxf, csz, dst_list):
                """PE-transpose idxf [csz, 32] halves; dst_list = list of
                (wrapped_tile, col_slice_for_even, col_slice_for_odd, src_cols)"""
                pT0 = psB.tile([16, 128], dt.float32, tag="B", name="pT0")
                pT1 = psB.tile([16, 128], dt.float32, tag="B", name="pT1")
                nc.tensor.transpose(pT0[:, :csz], idxf[:csz, 0:16],
                                    ident[:csz, :csz])
                nc.tensor.transpose(pT1[:, :csz], idxf[:csz, 16:32],
                                    ident[:csz, :csz])
                for wr, ev, od, (c0, cn) in dst_list:
                    w2 = wr[:].rearrange("p (i two) -> p i two", two=2)
                    nc.scalar.copy(w2[:, ev[0]:ev[0] + ev[1], 0],
                                   pT0[:, c0:c0 + cn])
                    nc.scalar.copy(w2[:, od[0]:od[0] + od[1], 1],
                                   pT1[:, c0:c0 + cn])

            def pair_stage(pr):
                xt = [xpool.tile([sz, 2 * ROI], dt.float32, tag=f"xt{i}", name=f"xt{i}")
                      for i, (o, sz) in enumerate(FCH)]
                for i, (o, sz) in enumerate(FCH):
                    nc.sync.dma_start(
                        xt[i][:], d_xT.ap()[o:o + sz,
                                            2 * ROI * pr:2 * ROI * (pr + 1)])
                # squared features + (-0.5) * column sums -> nsqrow
                sqt = [xpool.tile([sz, 2 * ROI], dt.float32, tag=f"sqt{i}", name=f"sqt{i}")
                       for i, (o, sz) in enumerate(FCH)]
                for i in range(3):
                    nc.scalar.square(sqt[i][:], xt[i][:])
                nsqrow = spool.tile([1, 2 * ROI], dt.float32, tag="nsqrow", name="nsqrow")
                for h in range(2):
                    pnsq = psB.tile([1, ROI], dt.float32, tag="B", name="pnsq")
                    for i, (o, sz) in enumerate(FCH):
                        nc.tensor.matmul(pnsq[:], ones_col[:sz, :],
                                         sqt[i][:, ROI * h:ROI * (h + 1)],
                                         start=(i == 0), stop=(i == 2))
                    nc.scalar.activation(nsqrow[:, ROI * h:ROI * (h + 1)],
                                         pnsq[:], AF.Copy, scale=-0.5)

                for h in range(2):              # graphs gl = 2*pr + h
                    gl = 2 * pr + h
                    pk, q = gl // 4, gl % 4
                    # ---- projections [u1|v1|cc2|v2] ----
                    pproj = psA.tile([128, ROI], dt.float32, tag="A", name="pproj")
                    for i, (o, sz) in enumerate(FCH):
                        nc.tensor.matmul(pproj[:], wproj[i][:],
                                         xt[i][:, ROI * h:ROI * (h + 1)],
                                         start=(i == 0), stop=(i == 2))
                    for cv in range(2):
                        nc.scalar.activation(
                            Up[cv][pk][32 * q:32 * q + 32, :],
                            pproj[64 * cv:64 * cv + 32, :], AF.Identity,
                            bias=bpack[32 * q:32 * q + 32, cv:cv + 1])
                        nc.scalar.copy(Vp[cv][pk][32 * q:32 * q + 32, :],
                                       pproj[64 * cv + 32:64 * cv + 64, :])
                    # ---- u3/v3 ----
                    pproj3a = psB.tile([1, ROI], dt.float32, tag="B", name="pproj3a")
                    pproj3b = psB.tile([1, ROI], dt.float32, tag="B", name="pproj3b")
                    for i, (o, sz) in enumerate(FCH):
                        nc.tensor.matmul(pproj3a[:], wproj3[i][:, 0:1],
                                         xt[i][:, ROI * h:ROI * (h + 1)],
                                         start=(i == 0), stop=(i == 2))
                    for i, (o, sz) in enumerate(FCH):
                        nc.tensor.matmul(pproj3b[:], wproj3[i][:, 1:2],
                                         xt[i][:, ROI * h:ROI * (h + 1)],
                                         start=(i == 0), stop=(i == 2))
                    nc.scalar.copy(u3row[:, ROI * gl:ROI * (gl + 1)], pproj3a[:])
                    nc.scalar.copy(v3row[:, ROI * gl:ROI * (gl + 1)], pproj3b[:])
                    # ---- d2 key + top-32 per i-chunk ----
                    for ic, (io, isz) in enumerate([(0, 128), (128, 128),
                                                    (256, 12)]):
                        pkey = psA.tile([128, ROI], dt.float32, tag="A", name="pkey")
                        for i, (o, sz) in enumerate(FCH):
                            nc.tensor.matmul(
                                pkey[:isz, :],
                                xt[i][:, ROI * h + io:ROI * h + io + isz],
                                xt[i][:, ROI * h:ROI * (h + 1)],
                                start=(i == 0), stop=False)
                        nc.tensor.matmul(pkey[:isz, :], ones_row[:, :isz],
                                         nsqrow[:, ROI * h:ROI * (h + 1)],
                                         start=False, stop=True)
                        if ic < 2:
                            keyS = spool.tile([128, ROI], dt.float32, tag="keyS", name="keyS")
                            nc.scalar.copy(keyS[:], pkey[:])
                            if taps and gl == 0 and ic == 0:
                                nc.sync.dma_start(d_taps["tap_key0"].ap(), keyS[:])
                            idxf = topk32(keyS, 128, gl, ic)
                            idx_to_wrapped(
                                idxf, 128,
                                [(wrapped[gl], (128 * ic, 128), (128 * ic, 128),
                                  (0, 128))])
                        else:
                            rstage = spool.tile([12, ROI], dt.float32,
                                                tag="rstage", name="rstage")
                            nc.scalar.copy(rstage[:], pkey[:12, :])
                            nc.sync.dma_start(packR[12 * gl:12 * gl + 12, :],
                                              rstage[:])

            idx2sb = [ppool.tile([128, EG // 16], dt.int16, tag=f"idx2_{t}", name=f"idx2_{t}")
                      for t in range(PACKS)]

            # remainder rows topk ([96, ROI] packed, 12 rows per graph)
            def do_packR_topk():
              idxfR = topk32(packR, 96, -1, -1)
              pTR0 = psB.tile([16, 96], dt.float32, tag="B", name="pTR0")
              pTR1 = psB.tile([16, 96], dt.float32, tag="B", name="pTR1")
              nc.tensor.transpose(pTR0[:], idxfR[:96, 0:16], ident[:96, :96])
              nc.tensor.transpose(pTR1[:], idxfR[:96, 16:32], ident[:96, :96])
              for g in range(GPC):
                  w2 = wrapped[g][:].rearrange("p (i two) -> p i two", two=2)
                  nc.scalar.copy(w2[:, 256:268, 0], pTR0[:, 12 * g:12 * g + 12])
                  nc.scalar.copy(w2[:, 256:268, 1], pTR1[:, 12 * g:12 * g + 12])
              if taps:
                  wr0f = spool.tile([16, EG // 16], dt.float32, tag="wr0f", name="wr0f")
                  nc.vector.tensor_copy(wr0f[:], wrapped[0][:])
                  nc.sync.dma_start(d_taps["tap_wr0"].ap(), wr0f[:])

              # device-built gcn2 gather index packs
              for g in range(GPC):
                  pk, q = g // 4, g % 4
                  nc.sync.dma_start(idx2sb[pk][32 * q:32 * q + 16, :], wrapped[g][:])
                  nc.sync.dma_start(idx2sb[pk][32 * q + 16:32 * q + 32, :],
                                    wrapped[g][:])

            # ================================================================
            # Stage 3: edge stages (gcn1, gcn2) + gcn3
            # ================================================================
            def edge_gather(cv, pk):
                idxp = idx1sb if cv == 0 else idx2sb
                Gv = epool.tile([128, EG], dt.float32, tag="Gv", name="Gv")
                nc.gpsimd.ap_gather(Gv[:], Vp[cv][pk][:], idxp[pk][:],
                                    channels=128, num_elems=ROI, d=1,
                                    num_idxs=EG)
                return Gv

            def edge_compute(cv, pk, Gv):
                g3 = Gv[:].rearrange("p (i k) -> p i k", k=DEG)
                ub = Up[cv][pk][:].unsqueeze(2).broadcast_to([128, ROI, DEG])
                nc.vector.tensor_tensor(g3, g3, ub, op=OP.add)
                Gb = epool.tile([128, EG], dt.bfloat16, tag="Gb", name="Gb")
                nc.scalar.activation(Gb[:], Gv[:], AF.Prelu, alpha=SLOPE)
                for ec, (eo, en) in enumerate(ECH):
                    pm = psM.tile([128, 512], dt.float32, tag="M", name="pm")
                    nc.tensor.matmul(pm[:, :en], wbd[cv][:],
                                     Gb[:, eo:eo + en], start=True, stop=True)
                    nc.vector.tensor_reduce(
                        aT[cv][pk][:, eo // DEG:(eo + en) // DEG],
                        pm[:, :en].rearrange("p (i k) -> p i k", k=DEG),
                        axis=AX.X, op=OP.max)
                sc = 4 * cv + pk
                qc = 4 * cv + 2 + pk
                nc.vector.tensor_reduce(stats6[:, sc:sc + 1],
                                        aT[cv][pk][:], axis=AX.X, op=OP.add)
                nc.scalar.activation(
                    sq_scratch[:], aT[cv][pk][:], AF.Square,
                    accum_out=stats6[:, qc:qc + 1])

            pair_stage(0)
            pair_stage(1)
            gv00 = edge_gather(0, 0)
            pair_stage(2)
            pair_stage(3)
            gv01 = edge_gather(0, 1)
            edge_compute(0, 0, gv00)
            do_packR_topk()
            gv10 = edge_gather(1, 0)
            edge_compute(0, 1, gv01)
            gv11 = edge_gather(1, 1)
            edge_compute(1, 0, gv10)

            # ---- gcn3: dense masked max (no gather) ----
            pA3u = psB.tile([128, 24], dt.float32, tag="B", name="pA3u")
            for g in range(GPC):
                pv3f = psA.tile([128, ROI], dt.float32, tag="A", name="pv3f")
                nc.tensor.matmul(pv3f[:],
                                 ones_row[0:1, :],
                                 v3row[0:1, ROI * g:ROI * (g + 1)],
                                 start=True, stop=True)
                for c, csz in enumerate((128, 128, 12)):
                    col = 3 * g + c
                    nc.vector.tensor_tensor(
                        t3scr[0:csz, :],
                        mask3[0:csz, ROI * col:ROI * (col + 1)],
                        pv3f[0:csz, :], op=OP.add)
                    nc.vector.tensor_reduce(
                        A3all[0:csz, col:col + 1], t3scr[0:csz, :],
                        axis=AX.X, op=OP.max)
                    nc.tensor.matmul(
                        pA3u[0:csz, col:col + 1],
                        u3row[0:1, ROI * g + 128 * c:ROI * g + 128 * c + csz],
                        ones_row[0:1, 0:1],
                        start=True, stop=True)
            for g in range(GPC):
                for c, csz in enumerate((128, 128, 12)):
                    col = 3 * g + c
                    nc.vector.tensor_tensor(A3all[0:csz, col:col + 1],
                                            A3all[0:csz, col:col + 1],
                                            pA3u[0:csz, col:col + 1], op=OP.add)
            nc.vector.tensor_reduce(stats6[:, 8:9], A3all[:], axis=AX.X, op=OP.add)
            sq24 = spool.tile([128, 24], dt.float32, tag="sq24", name="sq24")
            nc.scalar.activation(sq24[:], A3all[:], AF.Square,
                                 accum_out=stats6[:, 9:10])

            edge_compute(1, 1, gv11)

            # fold partial stats into columns [s1 q1 s2 q2 s3 q3]
            stats_o = spool.tile([128, 6], dt.float32, tag="stats_o", name="stats_o")
            nc.vector.tensor_tensor(stats_o[:, 0:1], stats6[:, 0:1],
                                    stats6[:, 1:2], op=OP.add)
            nc.vector.tensor_tensor(stats_o[:, 1:2], stats6[:, 2:3],
                                    stats6[:, 3:4], op=OP.add)
            nc.vector.tensor_tensor(stats_o[:, 2:3], stats6[:, 4:5],
                                    stats6[:, 5:6], op=OP.add)
            nc.vector.tensor_tensor(stats_o[:, 3:4], stats6[:, 6:7],
                                    stats6[:, 7:8], op=OP.add)
            nc.vector.tensor_copy(stats_o[:, 4:5], stats6[:, 8:9])
            nc.vector.tensor_copy(stats_o[:, 5:6], stats6[:, 9:10])
            pfold = psB.tile([33, 6], dt.float32, tag="B", name="pfold")
            nc.tensor.matmul(pfold[:], foldM[:], stats_o[:], start=True, stop=True)
            statsloc = spool.tile([33, 6], dt.float32, tag="statsloc", name="statsloc")
            nc.scalar.copy(statsloc[:], pfold[:])

            # ================================================================
            # Stage 4: AllReduce stats; BN affine params; x-stage; pooling
            # ================================================================
            b_in = dpool.tile([33, 6], dt.float32)
            b_out = dpool.tile([33, 6], dt.float32)
            nc.sync.dma_start(b_in[:], statsloc[:])
            if single:
                nc.sync.dma_start(b_out[:], b_in[:])
            else:
                nc.gpsimd.collective_compute(
                    "AllReduce", OP.add, replica_groups=[list(range(NCORES))],
                    ins=[b_in[:]], outs=[b_out[:]])
            statsg = spool.tile([33, 6], dt.float32, tag="statsg", name="statsg")
            nc.sync.dma_start(statsg[:], b_out[:])
            if taps:
                nc.sync.dma_start(d_taps["tap_stats"].ap(), statsg[:])
            # move conv3 stats (cols 4,5 at partition 32) into cols 0,1
            nc.scalar.copy(statsg[32:33, 0:2], statsg[32:33, 4:6])
            scaled = spool.tile([33, 4], dt.float32, tag="scaled", name="scaled")
            nc.vector.tensor_scalar_mul(scaled[:], statsg[:, 0:4], 1.0 / NTOT)
            var = spool.tile([33, 2], dt.float32, tag="var", name="var")
            sA = spool.tile([33, 4], dt.float32, tag="sA", name="sA")  # [sA tA sB tB]
            mu = scaled[:].rearrange("p (c two) -> p c two", two=2)
            nc.vector.tensor_tensor(var[:], mu[:, :, 0], mu[:, :, 0], op=OP.mult)
            nc.vector.tensor_tensor(var[:], mu[:, :, 1], var[:], op=OP.subtract)
            nc.vector.tensor_scalar_max(var[:], var[:], 0.0)
            nc.vector.tensor_scalar_add(var[:], var[:], EPS)
            nc.scalar.sqrt(var[:], var[:])
            nc.vector.reciprocal(var[:], var[:])
            sA2 = sA[:].rearrange("p (c two) -> p c two", two=2)
            gbe2 = gbe[:].rearrange("p (c two) -> p c two", two=2)
            nc.vector.tensor_tensor(sA2[:, :, 0], gbe2[:, :, 0], var[:], op=OP.mult)
            nc.vector.tensor_tensor(sA2[:, :, 1], mu[:, :, 0], sA2[:, :, 0],
                                    op=OP.mult)
            nc.vector.tensor_tensor(sA2[:, :, 1], gbe2[:, :, 1], sA2[:, :, 1],
                                    op=OP.subtract)
            paff1 = psB.tile([128, 4], dt.float32, tag="B", name="paff1")
            paff3 = psB.tile([128, 2], dt.float32, tag="B", name="paff3")
            nc.tensor.matmul(paff1[:], selM[:, 0:128], sA[:], start=True, stop=True)
            nc.tensor.matmul(paff3[:], selM[:, 128:256], sA[:, 0:2],
                             start=True, stop=True)
            aff1 = spool.tile([128, 4], dt.float32, tag="aff1", name="aff1")
            aff3 = spool.tile([128, 2], dt.float32, tag="aff3", name="aff3")
            nc.scalar.copy(aff1[:], paff1[:])
            nc.scalar.copy(aff3[:], paff3[:])
            if taps:
                nc.sync.dma_start(d_taps["tap_aff1"].ap(), aff1[:])
                for cv in range(2):
                    for pk in range(PACKS):
                        nc.sync.dma_start(
                            d_taps[f"tap_a{cv + 1}T{pk}"].ap(), aT[cv][pk][:])
                nc.sync.dma_start(d_taps["tap_a3row"].ap(), a3row[:])

            pool4 = spool.tile([128, 4], dt.float32, tag="pool4", name="pool4")
            for cv in range(2):
                for pk in range(PACKS):
                    nc.scalar.activation(aT[cv][pk][:], aT[cv][pk][:],
                                         AF.Prelu, alpha=SLOPE,
                                         scale=aff1[:, 2 * cv:2 * cv + 1],
                                         bias=aff1[:, 2 * cv + 1:2 * cv + 2])
                    nc.vector.tensor_reduce(pool4[:, 2 * cv + pk:2 * cv + pk + 1],
                                            aT[cv][pk][:], axis=AX.X, op=OP.add)
            A3f = spool.tile([128, 24], dt.float32, tag="A3f", name="A3f")
            nc.scalar.activation(A3f[:], A3all[:], AF.Prelu, alpha=SLOPE,
                                 scale=aff3[:, 0:1], bias=aff3[:, 1:2])

            # ================================================================
            # Stage 5: assemble zT [332, 8], AllGather -> [332, 64]
            # ================================================================
            zT0 = spool.tile([64, GPC], dt.float32, tag="zT0", name="zT0")
            g_in = dpool.tile([332, GPC], dt.float32, name="g_in")
            for g in range(GPC):
                pk, q = g // 4, g % 4
                nc.sync.dma_start(zT0[0:32, g:g + 1],
                                  pool4[32 * q:32 * q + 32, pk:pk + 1])
                nc.sync.dma_start(zT0[32:64, g:g + 1],
                                  pool4[32 * q:32 * q + 32, 2 + pk:3 + pk])
                nc.sync.dma_start(g_in[64:192, g:g + 1],
                                  A3f[:, 3 * g:3 * g + 1])
                nc.sync.dma_start(g_in[192:320, g:g + 1],
                                  A3f[:, 3 * g + 1:3 * g + 2])
                nc.sync.dma_start(g_in[320:332, g:g + 1],
                                  A3f[0:12, 3 * g + 2:3 * g + 3])
            g_out = dpool.tile([NCORES, 332, GPC], dt.float32, name="g_out")
            nc.sync.dma_start(g_in[0:64, :], zT0[:])
            if single:
                for cc in range(NCORES):
                    nc.sync.dma_start(g_out[cc, :, :], g_in[:])
            else:
                nc.gpsimd.collective_compute(
                    "AllGather", OP.bypass, replica_groups=[list(range(NCORES))],
                    ins=[g_in[:]], outs=[g_out[:]])
            zAll = [spool.tile([sz, B], dt.float32, tag=f"zAll{i}", name=f"zAll{i}")
                    for i, (o, sz) in enumerate(MCH)]
            for i, (o, sz) in enumerate(MCH):
                nc.sync.dma_start(
                    zAll[i][:].rearrange("r (c j) -> r c j", j=GPC),
                    g_out[:, o:o + sz, :].transpose([1, 0, 2]))
            if taps:
                nc.sync.dma_start(d_taps["tap_za"].ap(), zAll[0][:])
                nc.sync.dma_start(d_taps["tap_zb"].ap(), zAll[1][:])
                nc.sync.dma_start(d_taps["tap_zc"].ap(), zAll[2][:])

            # ================================================================
            # Stage 6: readout MLP, replicated on every core
            # ================================================================
            def bn_affine_cols(pml, gbe_t, col0, tag):
                """pml psum [128, B]; returns (s, t) [128,1] tiles."""
                s_sum = spool.tile([128, 1], dt.float32, tag=f"{tag}_sum", name=f"{tag}_sum")
                q_sum = spool.tile([128, 1], dt.float32, tag=f"{tag}_q", name=f"{tag}_q")
                scr = spool.tile([128, B], dt.float32, tag="mlp_scr", name="mlp_scr")
                nc.vector.tensor_reduce(s_sum[:], pml[:], axis=AX.X, op=OP.add)
                nc.scalar.activation(scr[:], pml[:], AF.Square, accum_out=q_sum[:])
                nc.vector.tensor_scalar_mul(s_sum[:], s_sum[:], 1.0 / B)
                nc.vector.tensor_scalar_mul(q_sum[:], q_sum[:], 1.0 / B)
                v = spool.tile([128, 1], dt.float32, tag=f"{tag}_v", name=f"{tag}_v")
                nc.vector.tensor_tensor(v[:], s_sum[:], s_sum[:], op=OP.mult)
                nc.vector.tensor_tensor(v[:], q_sum[:], v[:], op=OP.subtract)
                nc.vector.tensor_scalar_add(v[:], v[:], EPS)
                nc.scalar.sqrt(v[:], v[:])
                nc.vector.reciprocal(v[:], v[:])
                s_t = spool.tile([128, 1], dt.float32, tag=f"{tag}_s", name=f"{tag}_s")
                t_t = spool.tile([128, 1], dt.float32, tag=f"{tag}_t", name=f"{tag}_t")
                nc.vector.tensor_tensor(s_t[:], gbe_t[:, col0:col0 + 1], v[:],
                                        op=OP.mult)
                nc.vector.tensor_tensor(t_t[:], s_sum[:], s_t[:], op=OP.mult)
                nc.vector.tensor_tensor(t_t[:], gbe_t[:, col0 + 1:col0 + 2],
                                        t_t[:], op=OP.subtract)
                return s_t, t_t

            z1T = [spool.tile([128, B], dt.float32, tag=f"z1T{m}", name=f"z1T{m}")
                   for m in range(2)]
            for m in range(2):
                ph1 = psM.tile([128, B], dt.float32, tag="M", name="ph1")
                for i, (o, sz) in enumerate(MCH):
                    nc.tensor.matmul(ph1[:], wl1[i][:, 128 * m:128 * m + 128],
                                     zAll[i][:], start=(i == 0), stop=(i == 2))
                s_t, t_t = bn_affine_cols(ph1[:], gbe4, 2 * m, f"bn4_{m}")
                nc.scalar.activation(z1T[m][:], ph1[:], AF.Prelu, alpha=SLOPE,
                                     scale=s_t[:], bias=t_t[:])
            ph2 = psM.tile([128, B], dt.float32, tag="M", name="ph2")
            for m in range(2):
                nc.tensor.matmul(ph2[:], wl2[m][:], z1T[m][:],
                                 start=(m == 0), stop=(m == 1))
            s_t, t_t = bn_affine_cols(ph2[:], gbe5, 0, "bn5")
            z2T = spool.tile([128, B], dt.float32, tag="z2T", name="z2T")
            nc.scalar.activation(z2T[:], ph2[:], AF.Prelu, alpha=SLOPE,
                                 scale=s_t[:], bias=t_t[:])
            pout = psM.tile([1, B], dt.float32, tag="M", name="pout")
            nc.tensor.matmul(pout[:], wl3[:], z2T[:], start=True, stop=True)
            out_sb = spool.tile([1, B], dt.float32, tag="out_sb", name="out_sb")
            nc.scalar.activation(out_sb[:], pout[:], AF.Identity, bias=bl3[:])
            nc.sync.dma_start(d_out.ap(), out_sb[:])

    nc.compile()
    return nc


def _host_prep(inputs):
    x = np.asarray(inputs["x"])
    ei = np.asarray(inputs["edge_index"])
    src = ei[0]
    tgt = ei[1]
    exp_tgt = np.repeat(np.arange(NTOT, dtype=np.int64), DEG)
    assert np.array_equal(tgt.astype(np.int64), exp_tgt), \
        "edge_index structure mismatch (expected DGCN static grid)"
    assert np.array_equal(src // ROI, tgt // ROI), "cross-graph edges found"
    src_local = (src % ROI).astype(np.int16).reshape(B, ROI, DEG)

    # wrapped edge lists: el [B, EG]; wr [B, 16, EG//16]
    el = src_local.reshape(B, EG)
    wr = el.reshape(B, EG // 16, 16).transpose(0, 2, 1)  # pos n=(col*16+p)

    W1a, W2a, W3 = (_fp32(inputs[k]) for k in ("W1a", "W2a", "W3"))
    A1, B1 = W1a[:F] - W1a[F:], W1a[F:]
    A2, B2 = W2a[:F] - W2a[F:], W2a[F:]
    A3, B3 = W3[:F] - W3[F:], W3[F:]

    wproj = _fp32(np.concatenate([A1, B1, A2, B2], axis=1))
    wproj3 = _fp32(np.concatenate([A3, B3], axis=1))

    import ml_dtypes

    def blockdiag(w):
        out = np.zeros((128, 128), np.float32)
        for q in range(4):
            out[32 * q:32 * q + 32, 32 * q:32 * q + 32] = w
        return out

    wbd1 = blockdiag(_fp32(inputs["W1b"])).astype(ml_dtypes.bfloat16)
    wbd2 = blockdiag(_fp32(inputs["W2b"])).astype(ml_dtypes.bfloat16)
    bpack = _fp32(np.stack([np.tile(inputs["b1a"], 4),
                            np.tile(inputs["b2a"], 4)], axis=1))
    ident = np.eye(128, dtype=np.float32)

    foldM = np.zeros((128, 33), np.float32)
    p = np.arange(128)
    foldM[p, p % 32] = 1.0
    foldM[:, 32] = 1.0
    sel32 = np.zeros((33, 128), np.float32)
    sel32[p % 32, p] = 1.0
    sel3 = np.zeros((33, 128), np.float32)
    sel3[32, :] = 1.0
    selM = _fp32(np.concatenate([sel32, sel3], axis=1))

    gbe = np.zeros((33, 4), np.float32)
    gbe[0:32, 0] = inputs["g1"]
    gbe[0:32, 1] = inputs["be1"]
    gbe[0:32, 2] = inputs["g2"]
    gbe[0:32, 3] = inputs["be2"]
    gbe[32, 0] = inputs["g3"][0]
    gbe[32, 1] = inputs["be3"][0]


    wl1 = _fp32(inputs["Wl1"]).copy()
    wl1[0:64, :] /= ROI          # mean pooling folded into the weights
    wl2 = _fp32(inputs["Wl2"])
    wl3 = _fp32(inputs["Wl3"])
    gbe4 = _fp32(np.stack([inputs["g4"][0:128], inputs["be4"][0:128],
                           inputs["g4"][128:256], inputs["be4"][128:256]],
                          axis=1))
    gbe5 = _fp32(np.stack([inputs["g5"], inputs["be5"]], axis=1))
    bl3 = _fp32(inputs["bl3"].reshape(1, 1))

    shared = dict(ident=ident, wproj=wproj, wproj3=wproj3, wbd1=wbd1,
                  wbd2=wbd2, bpack=bpack, foldM=foldM, selM=selM, gbe=gbe,
                  wl1=wl1, wl2=wl2, wl3=wl3, gbe4=gbe4,
                  gbe5=gbe5, bl3=bl3)

    in_maps = []
    for c in range(NCORES):
        gs = slice(GPC * c, GPC * (c + 1))
        xT = _fp32(x[NLOC * c:NLOC * (c + 1)].T)
        idx1 = np.empty((2 * 128, EG // 16), np.int16)
        for t in range(2):
            for qq in range(4):
                g = GPC * c + 4 * t + qq
                idx1[128 * t + 32 * qq:128 * t + 32 * qq + 16] = wr[g]
                idx1[128 * t + 32 * qq + 16:128 * t + 32 * qq + 32] = wr[g]
        mask3 = np.full((128, 24, ROI), -1e30, np.float32)
        for g in range(GPC):
            sl = el[GPC * c + g].reshape(ROI, DEG)
            for cc3, csz3 in enumerate((128, 128, 12)):
                for pp in range(csz3):
                    mask3[pp, 3 * g + cc3, sl[128 * cc3 + pp]] = 0.0
        mask3 = mask3.reshape(128, 24 * ROI).astype(ml_dtypes.bfloat16)
        m = dict(shared)
        m.update(xT=xT, idx1=np.ascontiguousarray(idx1),
                 mask3=np.ascontiguousarray(mask3))
        in_maps.append(m)
    return in_maps


def kernel(**inputs):
    from concourse.bass_utils import run_bass_kernel_spmd

    if "nc" not in _cache:
        _cache["nc"] = _build_program()
    nc = _cache["nc"]
    in_maps = _host_prep(inputs)
    trace = bool(int(os.environ.get("KERNEL_TRACE", "0")))
    tmpdir = os.environ.get("KERNEL_TMPDIR") or None
    if tmpdir:
        os.makedirs(tmpdir, exist_ok=True)
    res = run_bass_kernel_spmd(nc, in_maps, core_ids=list(range(NCORES)),
                               trace=trace, tmpdir=tmpdir)
    _cache["last_results"] = res
    out = res.results[0]["out"].reshape(B, 1)
    return np.ascontiguousarray(out, dtype=np.float32)



# revision 4
# speedup vs baseline: 1.0042x; 1.0042x over previous
"""DGCN (EdgeConv x2 + DynamicEdgeConv + readout MLP) on 8 TRN2 NeuronCores.

Sharding: graph-level data parallel. 64 graphs -> 8 cores x 8 graphs.
Within a core, graphs are processed as 2 "packs" of 4 graphs (4 x 32ch = 128
partitions). All activations live transposed (channels/features on the
partition axis, nodes/edges on the free axis) so that:
  - projections/Gram matrices are plain matmuls over the feature axis,
  - EdgeConv neighbor gathers are single GPSIMD ap_gather ops along the
    free axis (per-16-partition-group index lists),
  - per-edge MLPs are block-diagonal 128-contraction matmuls,
  - BatchNorm affine+LeakyReLU collapse into one ScalarE activation.
BatchNorm statistics are global over all 17152 nodes -> one tiny AllReduce;
the readout MLP (BN over the 64-graph batch) runs replicated on every core
after an AllGather of the 332-dim per-graph feature vectors.
"""

import os
import sys

sys.path.insert(0, "/opt/trn_rl_repo")

import numpy as np

B = 64
ROI = 268
F = 268
C = 32
K = 32
DEG = 32
NCORES = 8
GPC = B // NCORES          # graphs per core = 8
NLOC = GPC * ROI           # nodes per core = 2144
NTOT = B * ROI             # 17152
EG = ROI * DEG             # edges per graph = 8576
PACKS = 2                  # 4-graph packs per core
SLOPE = 0.33
EPS = 1e-5

_cache = {}


def _fp32(a):
    return np.ascontiguousarray(a, dtype=np.float32)


def _build_program():
    import concourse.bacc as bacc
    import concourse.tile as tile
    import concourse.mybir as mybir
    from concourse import bass

    dt = mybir.dt
    f32r = dt.float32r
    AF = mybir.ActivationFunctionType
    OP = mybir.AluOpType
    AX = mybir.AxisListType

    taps = False
    single = bool(int(os.environ.get("KERNEL_SINGLE", "0")))

    nc = bacc.Bacc("TRN2", target_bir_lowering=False, debug=False,
                   num_devices=1 if single else NCORES)

    # ---- DRAM I/O -------------------------------------------------------
    d_xT = nc.dram_tensor("xT", [F, NLOC], dt.float32, kind="ExternalInput")
    d_idx1 = nc.dram_tensor("idx1", [2 * 128, EG // 16], dt.int16, kind="ExternalInput")
    d_ident = nc.dram_tensor("ident", [128, 128], dt.float32, kind="ExternalInput")
    d_wproj = nc.dram_tensor("wproj", [F, 128], dt.float32, kind="ExternalInput")
    d_wproj3 = nc.dram_tensor("wproj3", [F, 2], dt.float32, kind="ExternalInput")
    d_wbd1 = nc.dram_tensor("wbd1", [128, 128], dt.bfloat16, kind="ExternalInput")
    d_wbd2 = nc.dram_tensor("wbd2", [128, 128], dt.bfloat16, kind="ExternalInput")
    d_bpack = nc.dram_tensor("bpack", [128, 2], dt.float32, kind="ExternalInput")
    d_fold = nc.dram_tensor("foldM", [128, 33], dt.float32, kind="ExternalInput")
    d_sel = nc.dram_tensor("selM", [33, 256], dt.float32, kind="ExternalInput")
    d_gbe = nc.dram_tensor("gbe", [33, 4], dt.float32, kind="ExternalInput")
    d_mask3 = nc.dram_tensor("mask3", [128, 24 * ROI], dt.bfloat16,
                             kind="ExternalInput")
    d_wl1 = nc.dram_tensor("wl1", [332, 256], dt.float32, kind="ExternalInput")
    d_wl2 = nc.dram_tensor("wl2", [256, 128], dt.float32, kind="ExternalInput")
    d_wl3 = nc.dram_tensor("wl3", [128, 1], dt.float32, kind="ExternalInput")
    d_gbe4 = nc.dram_tensor("gbe4", [128, 4], dt.float32, kind="ExternalInput")
    d_gbe5 = nc.dram_tensor("gbe5", [128, 2], dt.float32, kind="ExternalInput")
    d_bl3 = nc.dram_tensor("bl3", [1, 1], dt.float32, kind="ExternalInput")
    d_out = nc.dram_tensor("out", [1, B], dt.float32, kind="ExternalOutput")

    d_taps = {}
    if taps:
        for nm, shp in [("tap_a1T0", [128, ROI]), ("tap_a1T1", [128, ROI]),
                        ("tap_a2T0", [128, ROI]), ("tap_a2T1", [128, ROI]),
                        ("tap_a3row", [128, ROI]), ("tap_stats", [33, 6]),
                        ("tap_aff1", [128, 4]), ("tap_pool", [128, 4]),
                        ("tap_za", [128, 64]), ("tap_zb", [128, 64]), ("tap_zc", [76, 64]), ("tap_wr0", [16, EG // 16]),
                        ("tap_key0", [128, ROI])]:
            d_taps[nm] = nc.dram_tensor(nm, shp, dt.float32, kind="ExternalOutput")

    FCH = [(0, 128), (128, 128), (256, 12)]      # feature-axis chunks
    ECH = [(i * 512, 512) for i in range(16)] + [(16 * 512, EG - 16 * 512)]

    with tile.TileContext(nc) as tc:
        with tc.tile_pool(name="const", bufs=1) as wpool, \
             tc.tile_pool(name="persist", bufs=1) as ppool, \
             tc.tile_pool(name="xt", bufs=2) as xpool, \
             tc.tile_pool(name="edge", bufs=2) as epool, \
             tc.tile_pool(name="scratch", bufs=2) as spool, \
             tc.tile_pool(name="psA", bufs=2, space="PSUM") as psA, \
             tc.tile_pool(name="psB", bufs=2, space="PSUM") as psB, \
             tc.tile_pool(name="psM", bufs=3, space="PSUM") as psM, \
             tc.tile_pool(name="dram", bufs=1, space="DRAM") as dpool:

            # ---- constants to SBUF -------------------------------------
            def load(name, shape, dtype, src):
                t = wpool.tile(shape, dtype, tag=name)
                nc.sync.dma_start(t[:], src)
                return t

            ident = load("ident", [128, 128], dt.float32, d_ident.ap())
            wproj = [load(f"wproj{i}", [sz, 128], dt.float32,
                          d_wproj.ap()[o:o + sz, :]) for i, (o, sz) in enumerate(FCH)]
            wproj3 = [load(f"wproj3{i}", [sz, 2], dt.float32,
                           d_wproj3.ap()[o:o + sz, :]) for i, (o, sz) in enumerate(FCH)]
            wbd = [load("wbd1", [128, 128], dt.bfloat16, d_wbd1.ap()),
                   load("wbd2", [128, 128], dt.bfloat16, d_wbd2.ap())]
            bpack = load("bpack", [128, 2], dt.float32, d_bpack.ap())
            foldM = load("foldM", [128, 33], dt.float32, d_fold.ap())
            selM = load("selM", [33, 256], dt.float32, d_sel.ap())
            gbe = load("gbe", [33, 4], dt.float32, d_gbe.ap())
            mask3 = load("mask3", [128, 24 * ROI], dt.bfloat16, d_mask3.ap())
            MCH = [(0, 128), (128, 128), (256, 76)]   # 332 rows of wl1 / zT
            wl1 = [load(f"wl1_{i}", [sz, 256], dt.float32,
                        d_wl1.ap()[o:o + sz, :]) for i, (o, sz) in enumerate(MCH)]
            wl2 = [load(f"wl2_{i}", [128, 128], dt.float32,
                        d_wl2.ap()[128 * i:128 * i + 128, :]) for i in range(2)]
            wl3 = load("wl3", [128, 1], dt.float32, d_wl3.ap())
            gbe4 = load("gbe4", [128, 4], dt.float32, d_gbe4.ap())
            gbe5 = load("gbe5", [128, 2], dt.float32, d_gbe5.ap())
            bl3 = load("bl3", [1, 1], dt.float32, d_bl3.ap())
            idx1sb = [load(f"idx1_{t}", [128, EG // 16], dt.int16,
                           d_idx1.ap()[128 * t:128 * t + 128, :]) for t in range(2)]

            ones_col = wpool.tile([128, 1], dt.float32, tag="ones_col", name="ones_col")
            nc.vector.memset(ones_col[:], 1.0)
            ones_row = wpool.tile([1, 128], dt.float32, tag="ones_row", name="ones_row")
            nc.vector.memset(ones_row[:], 1.0)

            # ---- persistent per-core tensors ---------------------------
            Vp = [[ppool.tile([128, ROI], dt.float32, tag=f"V{cv}p{pk}", name=f"V{cv}p{pk}")
                   for pk in range(PACKS)] for cv in range(2)]
            Up = [[ppool.tile([128, ROI], dt.float32, tag=f"U{cv}p{pk}", name=f"U{cv}p{pk}")
                   for pk in range(PACKS)] for cv in range(2)]
            aT = [[ppool.tile([128, ROI], dt.float32, tag=f"a{cv}p{pk}", name=f"a{cv}p{pk}")
                   for pk in range(PACKS)] for cv in range(2)]
            u3row = ppool.tile([1, NLOC], dt.float32, tag="u3row", name="u3row")
            v3row = ppool.tile([1, NLOC], dt.float32, tag="v3row", name="v3row")
            A3all = ppool.tile([128, 24], dt.float32, tag="A3all", name="A3all")
            t3scr = ppool.tile([128, ROI], dt.float32, tag="t3scr", name="t3scr")
            packR = ppool.tile([96, ROI], dt.float32, tag="packR", name="packR")
            wrapped = [ppool.tile([16, EG // 16], dt.int16, tag=f"wr{g}", name=f"wr{g}")
                       for g in range(GPC)]
            stats6 = ppool.tile([128, 10], dt.float32, tag="stats6", name="stats6")
            nc.vector.memset(A3all[:], 0.0)
            sq_scratch = ppool.tile([128, ROI], dt.float32, tag="sq_scratch", name="sq_scratch")

            # ================================================================
            # Stage 1: per graph-pair: load xT, squares, projections, d2 + topk
            # ================================================================
            def topk32(keyS, csz, gl, ic):
                """keyS [csz<=128, ROI] f32 SBUF (destroyed). Writes wrapped[gl]
                columns for i-chunk ic (ic in 0,1) or returns idxf for packR."""
                idxu = spool.tile([128, K], dt.uint32, tag="idxu", name="idxu")
                for r in range(4):
                    m8 = spool.tile([128, 8], dt.float32, tag=f"m8_{r % 2}", name=f"m8_{r % 2}")
                    nc.vector.max(m8[:csz, :], keyS[:csz, :])
                    nc.vector.max_index(idxu[:csz, 8 * r:8 * r + 8], m8[:csz, :],
                                        keyS[:csz, :])
                    if r < 3:
                        nc.vector.match_replace(keyS[:csz, :], m8[:csz, :],
                                                keyS[:csz, :], -1e30)
                idxf = spool.tile([128, K], dt.float32, tag="idxf", name="idxf")
                nc.vector.tensor_copy(idxf[:csz, :], idxu[:csz, :])
                return idxf

            def idx_to_wrapped(idxf, csz, dst_list):
                """PE-transpose idxf [csz, 32] halves; dst_list = list of
                (wrapped_tile, col_slice_for_even, col_slice_for_odd, src_cols)"""
                pT0 = psB.tile([16, 128], dt.float32, tag="B", name="pT0")
                pT1 = psB.tile([16, 128], dt.float32, tag="B", name="pT1")
                nc.tensor.transpose(pT0[:, :csz], idxf[:csz, 0:16],
                                    ident[:csz, :csz])
                nc.tensor.transpose(pT1[:, :csz], idxf[:csz, 16:32],
                                    ident[:csz, :csz])
                for wr, ev, od, (c0, cn) in dst_list:
                    w2 = wr[:].rearrange("p (i two) -> p i two", two=2)
                    nc.scalar.copy(w2[:, ev[0]:ev[0] + ev[1], 0],
                                   pT0[:, c0:c0 + cn])
                    nc.scalar.copy(w2[:, od[0]:od[0] + od[1], 1],
                                   pT1[:, c0:c0 + cn])

            def pair_stage(pr):
                xt = [xpool.tile([sz, 2 * ROI], dt.float32, tag=f"xt{i}", name=f"xt{i}")
                      for i, (o, sz) in enumerate(FCH)]
                for i, (o, sz) in enumerate(FCH):
                    nc.sync.dma_start(
                        xt[i][:], d_xT.ap()[o:o + sz,
                                            2 * ROI * pr:2 * ROI * (pr + 1)])
                # squared features + (-0.5) * column sums -> nsqrow
                sqt = [xpool.tile([sz, 2 * ROI], dt.float32, tag=f"sqt{i}", name=f"sqt{i}")
                       for i, (o, sz) in enumerate(FCH)]
                for i in range(3):
                    nc.scalar.square(sqt[i][:], xt[i][:])
                nsqrow = spool.tile([1, 2 * ROI], dt.float32, tag="nsqrow", name="nsqrow")
                for h in range(2):
                    pnsq = psB.tile([1, ROI], dt.float32, tag="B", name="pnsq")
                    for i, (o, sz) in enumerate(FCH):
                        nc.tensor.matmul(pnsq[:], ones_col[:sz, :],
                                         sqt[i][:, ROI * h:ROI * (h + 1)],
                                         start=(i == 0), stop=(i == 2))
                    nc.scalar.activation(nsqrow[:, ROI * h:ROI * (h + 1)],
                                         pnsq[:], AF.Copy, scale=-0.5)

                for h in range(2):              # graphs gl = 2*pr + h
                    gl = 2 * pr + h
                    pk, q = gl // 4, gl % 4
                    # ---- projections [u1|v1|cc2|v2] ----
                    pproj = psA.tile([128, ROI], dt.float32, tag="A", name="pproj")
                    for i, (o, sz) in enumerate(FCH):
                        nc.tensor.matmul(pproj[:], wproj[i][:],
                                         xt[i][:, ROI * h:ROI * (h + 1)],
                                         start=(i == 0), stop=(i == 2))
                    for cv in range(2):
                        nc.scalar.activation(
                            Up[cv][pk][32 * q:32 * q + 32, :],
                            pproj[64 * cv:64 * cv + 32, :], AF.Identity,
                            bias=bpack[32 * q:32 * q + 32, cv:cv + 1])
                        nc.scalar.copy(Vp[cv][pk][32 * q:32 * q + 32, :],
                                       pproj[64 * cv + 32:64 * cv + 64, :])
                    # ---- u3/v3 ----
                    pproj3a = psB.tile([1, ROI], dt.float32, tag="B", name="pproj3a")
                    pproj3b = psB.tile([1, ROI], dt.float32, tag="B", name="pproj3b")
                    for i, (o, sz) in enumerate(FCH):
                        nc.tensor.matmul(pproj3a[:], wproj3[i][:, 0:1],
                                         xt[i][:, ROI * h:ROI * (h + 1)],
                                         start=(i == 0), stop=(i == 2))
                    for i, (o, sz) in enumerate(FCH):
                        nc.tensor.matmul(pproj3b[:], wproj3[i][:, 1:2],
                                         xt[i][:, ROI * h:ROI * (h + 1)],
                                         start=(i == 0), stop=(i == 2))
                    nc.scalar.copy(u3row[:, ROI * gl:ROI * (gl + 1)], pproj3a[:])
                    nc.scalar.copy(v3row[:, ROI * gl:ROI * (gl + 1)], pproj3b[:])
                    # ---- d2 key + top-32 per i-chunk ----
                    for ic, (io, isz) in enumerate([(0, 128), (128, 128),
                                                    (256, 12)]):
                        pkey = psA.tile([128, ROI], dt.float32, tag="A", name="pkey")
                        for i, (o, sz) in enumerate(FCH):
                            nc.tensor.matmul(
                                pkey[:isz, :],
                                xt[i][:, ROI * h + io:ROI * h + io + isz],
                                xt[i][:, ROI * h:ROI * (h + 1)],
                                start=(i == 0), stop=False)
                        nc.tensor.matmul(pkey[:isz, :], ones_row[:, :isz],
                                         nsqrow[:, ROI * h:ROI * (h + 1)],
                                         start=False, stop=True)
                        if ic < 2:
                            keyS = spool.tile([128, ROI], dt.float32, tag="keyS", name="keyS")
                            nc.scalar.copy(keyS[:], pkey[:])
                            if taps and gl == 0 and ic == 0:
                                nc.sync.dma_start(d_taps["tap_key0"].ap(), keyS[:])
                            idxf = topk32(keyS, 128, gl, ic)
                            idx_to_wrapped(
                                idxf, 128,
                                [(wrapped[gl], (128 * ic, 128), (128 * ic, 128),
                                  (0, 128))])
                        else:
                            rstage = spool.tile([12, ROI], dt.float32,
                                                tag="rstage", name="rstage")
                            nc.scalar.copy(rstage[:], pkey[:12, :])
                            nc.sync.dma_start(packR[12 * gl:12 * gl + 12, :],
                                              rstage[:])

            idx2sb = [ppool.tile([128, EG // 16], dt.int16, tag=f"idx2_{t}", name=f"idx2_{t}")
                      for t in range(PACKS)]

            # remainder rows topk ([96, ROI] packed, 12 rows per graph)
            def do_packR_topk():
              idxfR = topk32(packR, 96, -1, -1)
              pTR0 = psB.tile([16, 96], dt.float32, tag="B", name="pTR0")
              pTR1 = psB.tile([16, 96], dt.float32, tag="B", name="pTR1")
              nc.tensor.transpose(pTR0[:], idxfR[:96, 0:16], ident[:96, :96])
              nc.tensor.transpose(pTR1[:], idxfR[:96, 16:32], ident[:96, :96])
              for g in range(GPC):
                  w2 = wrapped[g][:].rearrange("p (i two) -> p i two", two=2)
                  nc.scalar.copy(w2[:, 256:268, 0], pTR0[:, 12 * g:12 * g + 12])
                  nc.scalar.copy(w2[:, 256:268, 1], pTR1[:, 12 * g:12 * g + 12])
              if taps:
                  wr0f = spool.tile([16, EG // 16], dt.float32, tag="wr0f", name="wr0f")
                  nc.vector.tensor_copy(wr0f[:], wrapped[0][:])
                  nc.sync.dma_start(d_taps["tap_wr0"].ap(), wr0f[:])

              # device-built gcn2 gather index packs
              for g in range(GPC):
                  pk, q = g // 4, g % 4
                  nc.sync.dma_start(idx2sb[pk][32 * q:32 * q + 16, :], wrapped[g][:])
                  nc.sync.dma_start(idx2sb[pk][32 * q + 16:32 * q + 32, :],
                                    wrapped[g][:])

            # ================================================================
            # Stage 3: edge stages (gcn1, gcn2) + gcn3
            # ================================================================
            def edge_gather(cv, pk):
                idxp = idx1sb if cv == 0 else idx2sb
                Gv = epool.tile([128, EG], dt.float32, tag="Gv", name="Gv")
                nc.gpsimd.ap_gather(Gv[:], Vp[cv][pk][:], idxp[pk][:],
                                    channels=128, num_elems=ROI, d=1,
                                    num_idxs=EG)
                return Gv

            def edge_compute(cv, pk, Gv):
                g3 = Gv[:].rearrange("p (i k) -> p i k", k=DEG)
                ub = Up[cv][pk][:].unsqueeze(2).broadcast_to([128, ROI, DEG])
                nc.vector.tensor_tensor(g3, g3, ub, op=OP.add)
                Gb = epool.tile([128, EG], dt.bfloat16, tag="Gb", name="Gb")
                nc.scalar.activation(Gb[:], Gv[:], AF.Prelu, alpha=SLOPE)
                for ec, (eo, en) in enumerate(ECH):
                    pm = psM.tile([128, 512], dt.float32, tag="M", name="pm")
                    nc.tensor.matmul(pm[:, :en], wbd[cv][:],
                                     Gb[:, eo:eo + en], start=True, stop=True)
                    nc.vector.tensor_reduce(
                        aT[cv][pk][:, eo // DEG:(eo + en) // DEG],
                        pm[:, :en].rearrange("p (i k) -> p i k", k=DEG),
                        axis=AX.X, op=OP.max)
                sc = 4 * cv + pk
                qc = 4 * cv + 2 + pk
                nc.vector.tensor_reduce(stats6[:, sc:sc + 1],
                                        aT[cv][pk][:], axis=AX.X, op=OP.add)
                nc.scalar.activation(
                    sq_scratch[:], aT[cv][pk][:], AF.Square,
                    accum_out=stats6[:, qc:qc + 1])

            # CC-stream warm-up: dummy AllReduce on scratch, hidden under stage 1
            warm_in = dpool.tile([33, 6], dt.float32, name="warm_in")
            warm_out = dpool.tile([33, 6], dt.float32, name="warm_out")
            warmsrc = spool.tile([33, 6], dt.float32, tag="warmsrc", name="warmsrc")
            nc.vector.memset(warmsrc[:], 0.0)
            nc.sync.dma_start(warm_in[:], warmsrc[:])
            if not single:
                nc.gpsimd.collective_compute(
                    "AllReduce", OP.add, replica_groups=[list(range(NCORES))],
                    ins=[warm_in[:]], outs=[warm_out[:]])

            pair_stage(0)
            pair_stage(1)
            gv00 = edge_gather(0, 0)
            pair_stage(2)
            pair_stage(3)
            gv01 = edge_gather(0, 1)
            do_packR_topk()
            edge_compute(0, 0, gv00)
            gv10 = edge_gather(1, 0)
            edge_compute(0, 1, gv01)
            gv11 = edge_gather(1, 1)
            edge_compute(1, 0, gv10)

            # ---- gcn3: dense masked max (no gather) ----
            pA3u = psB.tile([128, 24], dt.float32, tag="B", name="pA3u")
            for g in range(GPC):
                pv3f = psA.tile([128, ROI], dt.float32, tag="A", name="pv3f")
                nc.tensor.matmul(pv3f[:],
                                 ones_row[0:1, :],
                                 v3row[0:1, ROI * g:ROI * (g + 1)],
                                 start=True, stop=True)
                for c, csz in enumerate((128, 128, 12)):
                    col = 3 * g + c
                    nc.vector.tensor_tensor(
                        t3scr[0:csz, :],
                        mask3[0:csz, ROI * col:ROI * (col + 1)],
                        pv3f[0:csz, :], op=OP.add)
                    nc.vector.tensor_reduce(
                        A3all[0:csz, col:col + 1], t3scr[0:csz, :],
                        axis=AX.X, op=OP.max)
                    nc.tensor.matmul(
                        pA3u[0:csz, col:col + 1],
                        u3row[0:1, ROI * g + 128 * c:ROI * g + 128 * c + csz],
                        ones_row[0:1, 0:1],
                        start=True, stop=True)
            for g in range(GPC):
                for c, csz in enumerate((128, 128, 12)):
                    col = 3 * g + c
                    nc.vector.tensor_tensor(A3all[0:csz, col:col + 1],
                                            A3all[0:csz, col:col + 1],
                                            pA3u[0:csz, col:col + 1], op=OP.add)
            nc.vector.tensor_reduce(stats6[:, 8:9], A3all[:], axis=AX.X, op=OP.add)
            sq24 = spool.tile([128, 24], dt.float32, tag="sq24", name="sq24")
            nc.scalar.activation(sq24[:], A3all[:], AF.Square,
                                 accum_out=stats6[:, 9:10])

            edge_compute(1, 1, gv11)

            # fold partial stats into columns [s1 q1 s2 q2 s3 q3]
            stats_o = spool.tile([128, 6], dt.float32, tag="stats_o", name="stats_o")
            nc.vector.tensor_tensor(stats_o[:, 0:1], stats6[:, 0:1],
                                    stats6[:, 1:2], op=OP.add)
            nc.vector.tensor_tensor(stats_o[:, 1:2], stats6[:, 2:3],
                                    stats6[:, 3:4], op=OP.add)
            nc.vector.tensor_tensor(stats_o[:, 2:3], stats6[:, 4:5],
                                    stats6[:, 5:6], op=OP.add)
            nc.vector.tensor_tensor(stats_o[:, 3:4], stats6[:, 6:7],
                                    stats6[:, 7:8], op=OP.add)
            nc.vector.tensor_copy(stats_o[:, 4:5], stats6[:, 8:9])
            nc.vector.tensor_copy(stats_o[:, 5:6], stats6[:, 9:10])
            pfold = psB.tile([33, 6], dt.float32, tag="B", name="pfold")
            nc.tensor.matmul(pfold[:], foldM[:], stats_o[:], start=True, stop=True)
            statsloc = spool.tile([33, 6], dt.float32, tag="statsloc", name="statsloc")
            nc.scalar.copy(statsloc[:], pfold[:])

            # ================================================================
            # Stage 4: AllReduce stats; BN affine params; x-stage; pooling
            # ================================================================
            b_in = dpool.tile([33, 6], dt.float32)
            b_out = dpool.tile([33, 6], dt.float32)
            nc.sync.dma_start(b_in[:], statsloc[:])
            if single:
                nc.sync.dma_start(b_out[:], b_in[:])
            else:
                nc.gpsimd.collective_compute(
                    "AllReduce", OP.add, replica_groups=[list(range(NCORES))],
                    ins=[b_in[:]], outs=[b_out[:]])
            statsg = spool.tile([33, 6], dt.float32, tag="statsg", name="statsg")
            nc.sync.dma_start(statsg[:], b_out[:])
            if taps:
                nc.sync.dma_start(d_taps["tap_stats"].ap(), statsg[:])
            # move conv3 stats (cols 4,5 at partition 32) into cols 0,1
            nc.scalar.copy(statsg[32:33, 0:2], statsg[32:33, 4:6])
            scaled = spool.tile([33, 4], dt.float32, tag="scaled", name="scaled")
            nc.vector.tensor_scalar_mul(scaled[:], statsg[:, 0:4], 1.0 / NTOT)
            var = spool.tile([33, 2], dt.float32, tag="var", name="var")
            sA = spool.tile([33, 4], dt.float32, tag="sA", name="sA")  # [sA tA sB tB]
            mu = scaled[:].rearrange("p (c two) -> p c two", two=2)
            nc.vector.tensor_tensor(var[:], mu[:, :, 0], mu[:, :, 0], op=OP.mult)
            nc.vector.tensor_tensor(var[:], mu[:, :, 1], var[:], op=OP.subtract)
            nc.vector.tensor_scalar_max(var[:], var[:], 0.0)
            nc.vector.tensor_scalar_add(var[:], var[:], EPS)
            nc.scalar.sqrt(var[:], var[:])
            nc.vector.reciprocal(var[:], var[:])
            sA2 = sA[:].rearrange("p (c two) -> p c two", two=2)
            gbe2 = gbe[:].rearrange("p (c two) -> p c two", two=2)
            nc.vector.tensor_tensor(sA2[:, :, 0], gbe2[:, :, 0], var[:], op=OP.mult)
            nc.vector.tensor_tensor(sA2[:, :, 1], mu[:, :, 0], sA2[:, :, 0],
                                    op=OP.mult)
            nc.vector.tensor_tensor(sA2[:, :, 1], gbe2[:, :, 1], sA2[:, :, 1],
                                    op=OP.subtract)
            paff1 = psB.tile([128, 4], dt.float32, tag="B", name="paff1")
            paff3 = psB.tile([128, 2], dt.float32, tag="B", name="paff3")
            nc.tensor.matmul(paff1[:], selM[:, 0:128], sA[:], start=True, stop=True)
            nc.tensor.matmul(paff3[:], selM[:, 128:256], sA[:, 0:2],
                             start=True, stop=True)
            aff1 = spool.tile([128, 4], dt.float32, tag="aff1", name="aff1")
            aff3 = spool.tile([128, 2], dt.float32, tag="aff3", name="aff3")
            nc.scalar.copy(aff1[:], paff1[:])
            nc.scalar.copy(aff3[:], paff3[:])
            if taps:
                nc.sync.dma_start(d_taps["tap_aff1"].ap(), aff1[:])
                for cv in range(2):
                    for pk in range(PACKS):
                        nc.sync.dma_start(
                            d_taps[f"tap_a{cv + 1}T{pk}"].ap(), aT[cv][pk][:])
                nc.sync.dma_start(d_taps["tap_a3row"].ap(), a3row[:])

            pool4 = spool.tile([128, 4], dt.float32, tag="pool4", name="pool4")
            for cv in range(2):
                for pk in range(PACKS):
                    nc.scalar.activation(aT[cv][pk][:], aT[cv][pk][:],
                                         AF.Prelu, alpha=SLOPE,
                                         scale=aff1[:, 2 * cv:2 * cv + 1],
                                         bias=aff1[:, 2 * cv + 1:2 * cv + 2])
                    nc.vector.tensor_reduce(pool4[:, 2 * cv + pk:2 * cv + pk + 1],
                                            aT[cv][pk][:], axis=AX.X, op=OP.add)
            A3f = spool.tile([128, 24], dt.float32, tag="A3f", name="A3f")
            nc.scalar.activation(A3f[:], A3all[:], AF.Prelu, alpha=SLOPE,
                                 scale=aff3[:, 0:1], bias=aff3[:, 1:2])

            # ================================================================
            # Stage 5: assemble zT [332, 8], AllGather -> [332, 64]
            # ================================================================
            zT0 = spool.tile([64, GPC], dt.float32, tag="zT0", name="zT0")
            g_in = dpool.tile([332, GPC], dt.float32, name="g_in")
            for g in range(GPC):
                pk, q = g // 4, g % 4
                nc.sync.dma_start(zT0[0:32, g:g + 1],
                                  pool4[32 * q:32 * q + 32, pk:pk + 1])
                nc.sync.dma_start(zT0[32:64, g:g + 1],
                                  pool4[32 * q:32 * q + 32, 2 + pk:3 + pk])
                nc.sync.dma_start(g_in[64:192, g:g + 1],
                                  A3f[:, 3 * g:3 * g + 1])
                nc.sync.dma_start(g_in[192:320, g:g + 1],
                                  A3f[:, 3 * g + 1:3 * g + 2])
                nc.sync.dma_start(g_in[320:332, g:g + 1],
                                  A3f[0:12, 3 * g + 2:3 * g + 3])
            g_out = dpool.tile([NCORES, 332, GPC], dt.float32, name="g_out")
            nc.sync.dma_start(g_in[0:64, :], zT0[:])
            if single:
                for cc in range(NCORES):
                    nc.sync.dma_start(g_out[cc, :, :], g_in[:])
            else:
                nc.gpsimd.collective_compute(
                    "AllGather", OP.bypass, replica_groups=[list(range(NCORES))],
                    ins=[g_in[:]], outs=[g_out[:]])
            zAll = [spool.tile([sz, B], dt.float32, tag=f"zAll{i}", name=f"zAll{i}")
                    for i, (o, sz) in enumerate(MCH)]
            for i, (o, sz) in enumerate(MCH):
                nc.sync.dma_start(
                    zAll[i][:].rearrange("r (c j) -> r c j", j=GPC),
                    g_out[:, o:o + sz, :].transpose([1, 0, 2]))
            if taps:
                nc.sync.dma_start(d_taps["tap_za"].ap(), zAll[0][:])
                nc.sync.dma_start(d_taps["tap_zb"].ap(), zAll[1][:])
                nc.sync.dma_start(d_taps["tap_zc"].ap(), zAll[2][:])

            # ================================================================
            # Stage 6: readout MLP, replicated on every core
            # ================================================================
            def bn_affine_cols(pml, gbe_t, col0, tag):
                """pml psum [128, B]; returns (s, t) [128,1] tiles."""
                s_sum = spool.tile([128, 1], dt.float32, tag=f"{tag}_sum", name=f"{tag}_sum")
                q_sum = spool.tile([128, 1], dt.float32, tag=f"{tag}_q", name=f"{tag}_q")
                scr = spool.tile([128, B], dt.float32, tag="mlp_scr", name="mlp_scr")
                nc.vector.tensor_reduce(s_sum[:], pml[:], axis=AX.X, op=OP.add)
                nc.scalar.activation(scr[:], pml[:], AF.Square, accum_out=q_sum[:])
                nc.vector.tensor_scalar_mul(s_sum[:], s_sum[:], 1.0 / B)
                nc.vector.tensor_scalar_mul(q_sum[:], q_sum[:], 1.0 / B)
                v = spool.tile([128, 1], dt.float32, tag=f"{tag}_v", name=f"{tag}_v")
                nc.vector.tensor_tensor(v[:], s_sum[:], s_sum[:], op=OP.mult)
                nc.vector.tensor_tensor(v[:], q_sum[:], v[:], op=OP.subtract)
                nc.vector.tensor_scalar_add(v[:], v[:], EPS)
                nc.scalar.sqrt(v[:], v[:])
                nc.vector.reciprocal(v[:], v[:])
                s_t = spool.tile([128, 1], dt.float32, tag=f"{tag}_s", name=f"{tag}_s")
                t_t = spool.tile([128, 1], dt.float32, tag=f"{tag}_t", name=f"{tag}_t")
                nc.vector.tensor_tensor(s_t[:], gbe_t[:, col0:col0 + 1], v[:],
                                        op=OP.mult)
                nc.vector.tensor_tensor(t_t[:], s_sum[:], s_t[:], op=OP.mult)
                nc.vector.tensor_tensor(t_t[:], gbe_t[:, col0 + 1:col0 + 2],
                                        t_t[:], op=OP.subtract)
                return s_t, t_t

            z1T = [spool.tile([128, B], dt.float32, tag=f"z1T{m}", name=f"z1T{m}")
                   for m in range(2)]
            for m in range(2):
                ph1 = psM.tile([128, B], dt.float32, tag="M", name="ph1")
                for i, (o, sz) in enumerate(MCH):
                    nc.tensor.matmul(ph1[:], wl1[i][:, 128 * m:128 * m + 128],
                                     zAll[i][:], start=(i == 0), stop=(i == 2))
                s_t, t_t = bn_affine_cols(ph1[:], gbe4, 2 * m, f"bn4_{m}")
                nc.scalar.activation(z1T[m][:], ph1[:], AF.Prelu, alpha=SLOPE,
                                     scale=s_t[:], bias=t_t[:])
            ph2 = psM.tile([128, B], dt.float32, tag="M", name="ph2")
            for m in range(2):
                nc.tensor.matmul(ph2[:], wl2[m][:], z1T[m][:],
                                 start=(m == 0), stop=(m == 1))
            s_t, t_t = bn_affine_cols(ph2[:], gbe5, 0, "bn5")
            z2T = spool.tile([128, B], dt.float32, tag="z2T", name="z2T")
            nc.scalar.activation(z2T[:], ph2[:], AF.Prelu, alpha=SLOPE,
                                 scale=s_t[:], bias=t_t[:])
            pout = psM.tile([1, B], dt.float32, tag="M", name="pout")
            nc.tensor.matmul(pout[:], wl3[:], z2T[:], start=True, stop=True)
            out_sb = spool.tile([1, B], dt.float32, tag="out_sb", name="out_sb")
            nc.scalar.activation(out_sb[:], pout[:], AF.Identity, bias=bl3[:])
            nc.sync.dma_start(d_out.ap(), out_sb[:])

    nc.compile()
    return nc


def _host_prep(inputs):
    x = np.asarray(inputs["x"])
    ei = np.asarray(inputs["edge_index"])
    src = ei[0]
    tgt = ei[1]
    exp_tgt = np.repeat(np.arange(NTOT, dtype=np.int64), DEG)
    assert np.array_equal(tgt.astype(np.int64), exp_tgt), \
        "edge_index structure mismatch (expected DGCN static grid)"
    assert np.array_equal(src // ROI, tgt // ROI), "cross-graph edges found"
    src_local = (src % ROI).astype(np.int16).reshape(B, ROI, DEG)

    # wrapped edge lists: el [B, EG]; wr [B, 16, EG//16]
    el = src_local.reshape(B, EG)
    wr = el.reshape(B, EG // 16, 16).transpose(0, 2, 1)  # pos n=(col*16+p)

    W1a, W2a, W3 = (_fp32(inputs[k]) for k in ("W1a", "W2a", "W3"))
    A1, B1 = W1a[:F] - W1a[F:], W1a[F:]
    A2, B2 = W2a[:F] - W2a[F:], W2a[F:]
    A3, B3 = W3[:F] - W3[F:], W3[F:]

    wproj = _fp32(np.concatenate([A1, B1, A2, B2], axis=1))
    wproj3 = _fp32(np.concatenate([A3, B3], axis=1))

    import ml_dtypes

    def blockdiag(w):
        out = np.zeros((128, 128), np.float32)
        for q in range(4):
            out[32 * q:32 * q + 32, 32 * q:32 * q + 32] = w
        return out

    wbd1 = blockdiag(_fp32(inputs["W1b"])).astype(ml_dtypes.bfloat16)
    wbd2 = blockdiag(_fp32(inputs["W2b"])).astype(ml_dtypes.bfloat16)
    bpack = _fp32(np.stack([np.tile(inputs["b1a"], 4),
                            np.tile(inputs["b2a"], 4)], axis=1))
    ident = np.eye(128, dtype=np.float32)

    foldM = np.zeros((128, 33), np.float32)
    p = np.arange(128)
    foldM[p, p % 32] = 1.0
    foldM[:, 32] = 1.0
    sel32 = np.zeros((33, 128), np.float32)
    sel32[p % 32, p] = 1.0
    sel3 = np.zeros((33, 128), np.float32)
    sel3[32, :] = 1.0
    selM = _fp32(np.concatenate([sel32, sel3], axis=1))

    gbe = np.zeros((33, 4), np.float32)
    gbe[0:32, 0] = inputs["g1"]
    gbe[0:32, 1] = inputs["be1"]
    gbe[0:32, 2] = inputs["g2"]
    gbe[0:32, 3] = inputs["be2"]
    gbe[32, 0] = inputs["g3"][0]
    gbe[32, 1] = inputs["be3"][0]


    wl1 = _fp32(inputs["Wl1"]).copy()
    wl1[0:64, :] /= ROI          # mean pooling folded into the weights
    wl2 = _fp32(inputs["Wl2"])
    wl3 = _fp32(inputs["Wl3"])
    gbe4 = _fp32(np.stack([inputs["g4"][0:128], inputs["be4"][0:128],
                           inputs["g4"][128:256], inputs["be4"][128:256]],
                          axis=1))
    gbe5 = _fp32(np.stack([inputs["g5"], inputs["be5"]], axis=1))
    bl3 = _fp32(inputs["bl3"].reshape(1, 1))

    shared = dict(ident=ident, wproj=wproj, wproj3=wproj3, wbd1=wbd1,
                  wbd2=wbd2, bpack=bpack, foldM=foldM, selM=selM, gbe=gbe,
                  wl1=wl1, wl2=wl2, wl3=wl3, gbe4=gbe4,
                  gbe5=gbe5, bl3=bl3)

    in_maps = []
    for c in range(NCORES):
        gs = slice(GPC * c, GPC * (c + 1))
        xT = _fp32(x[NLOC * c:NLOC * (c + 1)].T)
        idx1 = np.empty((2 * 128, EG // 16), np.int16)
        for t in range(2):
            for qq in range(4):
                g = GPC * c + 4 * t + qq
                idx1[128 * t + 32 * qq:128 * t + 32 * qq + 16] = wr[g]
                idx1[128 * t + 32 * qq + 16:128 * t + 32 * qq + 32] = wr[g]
        mask3 = np.full((128, 24, ROI), -1e30, np.float32)
        for g in range(GPC):
            sl = el[GPC * c + g].reshape(ROI, DEG)
            for cc3, csz3 in enumerate((128, 128, 12)):
                for pp in range(csz3):
                    mask3[pp, 3 * g + cc3, sl[128 * cc3 + pp]] = 0.0
        mask3 = mask3.reshape(128, 24 * ROI).astype(ml_dtypes.bfloat16)
        m = dict(shared)
        m.update(xT=xT, idx1=np.ascontiguousarray(idx1),
                 mask3=np.ascontiguousarray(mask3))
        in_maps.append(m)
    return in_maps


def kernel(**inputs):
    from concourse.bass_utils import run_bass_kernel_spmd

    if "nc" not in _cache:
        _cache["nc"] = _build_program()
    nc = _cache["nc"]
    in_maps = _host_prep(inputs)
    trace = bool(int(os.environ.get("KERNEL_TRACE", "0")))
    tmpdir = os.environ.get("KERNEL_TMPDIR") or None
    if tmpdir:
        os.makedirs(tmpdir, exist_ok=True)
    res = run_bass_kernel_spmd(nc, in_maps, core_ids=list(range(NCORES)),
                               trace=trace, tmpdir=tmpdir)
    _cache["last_results"] = res
    out = res.results[0]["out"].reshape(B, 1)
    return np.ascontiguousarray(out, dtype=np.float32)



# revision 5
# speedup vs baseline: 1.0398x; 1.0355x over previous
"""DGCN (EdgeConv x2 + DynamicEdgeConv + readout MLP) on 8 TRN2 NeuronCores.

Sharding: graph-level data parallel. 64 graphs -> 8 cores x 8 graphs.
Within a core, graphs are processed as 2 "packs" of 4 graphs (4 x 32ch = 128
partitions). All activations live transposed (channels/features on the
partition axis, nodes/edges on the free axis) so that:
  - projections/Gram matrices are plain matmuls over the feature axis,
  - EdgeConv neighbor gathers are single GPSIMD ap_gather ops along the
    free axis (per-16-partition-group index lists),
  - per-edge MLPs are block-diagonal 128-contraction matmuls,
  - BatchNorm affine+LeakyReLU collapse into one ScalarE activation.
BatchNorm statistics are global over all 17152 nodes -> one tiny AllReduce;
the readout MLP (BN over the 64-graph batch) runs replicated on every core
after an AllGather of the 332-dim per-graph feature vectors.
"""

import os
import sys

sys.path.insert(0, "/opt/trn_rl_repo")

import numpy as np

B = 64
ROI = 268
F = 268
C = 32
K = 32
DEG = 32
NCORES = 8
GPC = B // NCORES          # graphs per core = 8
NLOC = GPC * ROI           # nodes per core = 2144
NTOT = B * ROI             # 17152
EG = ROI * DEG             # edges per graph = 8576
PACKS = 2                  # 4-graph packs per core
SLOPE = 0.33
EPS = 1e-5

_cache = {}


def _fp32(a):
    return np.ascontiguousarray(a, dtype=np.float32)


def _build_program():
    import concourse.bacc as bacc
    import concourse.tile as tile
    import concourse.mybir as mybir
    from concourse import bass

    dt = mybir.dt
    f32r = dt.float32r
    AF = mybir.ActivationFunctionType
    OP = mybir.AluOpType
    AX = mybir.AxisListType

    taps = False
    single = bool(int(os.environ.get("KERNEL_SINGLE", "0")))

    nc = bacc.Bacc("TRN2", target_bir_lowering=False, debug=False,
                   num_devices=1 if single else NCORES)

    # ---- DRAM I/O -------------------------------------------------------
    d_xT = nc.dram_tensor("xT", [F, NLOC], dt.float32, kind="ExternalInput")
    d_idx1 = nc.dram_tensor("idx1", [2 * 128, EG // 16], dt.int16, kind="ExternalInput")
    d_ident = nc.dram_tensor("ident", [128, 128], dt.float32, kind="ExternalInput")
    d_wproj = nc.dram_tensor("wproj", [F, 128], dt.float32, kind="ExternalInput")
    d_wproj3 = nc.dram_tensor("wproj3", [F, 2], dt.float32, kind="ExternalInput")
    d_wbd1 = nc.dram_tensor("wbd1", [128, 128], dt.bfloat16, kind="ExternalInput")
    d_wbd2 = nc.dram_tensor("wbd2", [128, 128], dt.bfloat16, kind="ExternalInput")
    d_bpack = nc.dram_tensor("bpack", [128, 2], dt.float32, kind="ExternalInput")
    d_fold = nc.dram_tensor("foldM", [128, 33], dt.float32, kind="ExternalInput")
    d_sel = nc.dram_tensor("selM", [33, 256], dt.float32, kind="ExternalInput")
    d_gbe = nc.dram_tensor("gbe", [33, 4], dt.float32, kind="ExternalInput")
    d_mask3 = nc.dram_tensor("mask3", [128, 24 * ROI], dt.bfloat16,
                             kind="ExternalInput")
    d_wl1 = nc.dram_tensor("wl1", [332, 256], dt.float32, kind="ExternalInput")
    d_wl2 = nc.dram_tensor("wl2", [256, 128], dt.float32, kind="ExternalInput")
    d_wl3 = nc.dram_tensor("wl3", [128, 1], dt.float32, kind="ExternalInput")
    d_gbe4 = nc.dram_tensor("gbe4", [128, 4], dt.float32, kind="ExternalInput")
    d_gbe5 = nc.dram_tensor("gbe5", [128, 2], dt.float32, kind="ExternalInput")
    d_bl3 = nc.dram_tensor("bl3", [1, 1], dt.float32, kind="ExternalInput")
    d_out = nc.dram_tensor("out", [1, B], dt.float32, kind="ExternalOutput")

    d_taps = {}
    if taps:
        for nm, shp in [("tap_a1T0", [128, ROI]), ("tap_a1T1", [128, ROI]),
                        ("tap_a2T0", [128, ROI]), ("tap_a2T1", [128, ROI]),
                        ("tap_a3row", [128, ROI]), ("tap_stats", [33, 6]),
                        ("tap_aff1", [128, 4]), ("tap_pool", [128, 4]),
                        ("tap_za", [128, 64]), ("tap_zb", [128, 64]), ("tap_zc", [76, 64]), ("tap_wr0", [16, EG // 16]),
                        ("tap_key0", [128, ROI])]:
            d_taps[nm] = nc.dram_tensor(nm, shp, dt.float32, kind="ExternalOutput")

    FCH = [(0, 128), (128, 128), (256, 12)]      # feature-axis chunks
    ECH = [(i * 512, 512) for i in range(16)] + [(16 * 512, EG - 16 * 512)]

    with tile.TileContext(nc) as tc:
        with tc.tile_pool(name="const", bufs=1) as wpool, \
             tc.tile_pool(name="persist", bufs=1) as ppool, \
             tc.tile_pool(name="xt", bufs=2) as xpool, \
             tc.tile_pool(name="edge", bufs=2) as epool, \
             tc.tile_pool(name="scratch", bufs=2) as spool, \
             tc.tile_pool(name="psA", bufs=2, space="PSUM") as psA, \
             tc.tile_pool(name="psB", bufs=2, space="PSUM") as psB, \
             tc.tile_pool(name="psM", bufs=3, space="PSUM") as psM, \
             tc.tile_pool(name="dram", bufs=1, space="DRAM") as dpool:

            # ---- constants to SBUF -------------------------------------
            def load(name, shape, dtype, src):
                t = wpool.tile(shape, dtype, tag=name)
                nc.sync.dma_start(t[:], src)
                return t

            ident = load("ident", [128, 128], dt.float32, d_ident.ap())
            wproj = [load(f"wproj{i}", [sz, 128], dt.float32,
                          d_wproj.ap()[o:o + sz, :]) for i, (o, sz) in enumerate(FCH)]
            wproj3 = [load(f"wproj3{i}", [sz, 2], dt.float32,
                           d_wproj3.ap()[o:o + sz, :]) for i, (o, sz) in enumerate(FCH)]
            wbd = [load("wbd1", [128, 128], dt.bfloat16, d_wbd1.ap()),
                   load("wbd2", [128, 128], dt.bfloat16, d_wbd2.ap())]
            bpack = load("bpack", [128, 2], dt.float32, d_bpack.ap())
            foldM = load("foldM", [128, 33], dt.float32, d_fold.ap())
            selM = load("selM", [33, 256], dt.float32, d_sel.ap())
            gbe = load("gbe", [33, 4], dt.float32, d_gbe.ap())
            mask3 = load("mask3", [128, 24 * ROI], dt.bfloat16, d_mask3.ap())
            MCH = [(0, 128), (128, 128), (256, 76)]   # 332 rows of wl1 / zT
            wl1 = [load(f"wl1_{i}", [sz, 256], dt.float32,
                        d_wl1.ap()[o:o + sz, :]) for i, (o, sz) in enumerate(MCH)]
            wl2 = [load(f"wl2_{i}", [128, 128], dt.float32,
                        d_wl2.ap()[128 * i:128 * i + 128, :]) for i in range(2)]
            wl3 = load("wl3", [128, 1], dt.float32, d_wl3.ap())
            gbe4 = load("gbe4", [128, 4], dt.float32, d_gbe4.ap())
            gbe5 = load("gbe5", [128, 2], dt.float32, d_gbe5.ap())
            bl3 = load("bl3", [1, 1], dt.float32, d_bl3.ap())
            idx1sb = [load(f"idx1_{t}", [128, EG // 16], dt.int16,
                           d_idx1.ap()[128 * t:128 * t + 128, :]) for t in range(2)]

            ones_col = wpool.tile([128, 1], dt.float32, tag="ones_col", name="ones_col")
            nc.vector.memset(ones_col[:], 1.0)
            ones_row = wpool.tile([1, 128], dt.float32, tag="ones_row", name="ones_row")
            nc.vector.memset(ones_row[:], 1.0)

            # ---- persistent per-core tensors ---------------------------
            Vp = [[ppool.tile([128, ROI], dt.float32, tag=f"V{cv}p{pk}", name=f"V{cv}p{pk}")
                   for pk in range(PACKS)] for cv in range(2)]
            Up = [[ppool.tile([128, ROI], dt.float32, tag=f"U{cv}p{pk}", name=f"U{cv}p{pk}")
                   for pk in range(PACKS)] for cv in range(2)]
            aT = [[ppool.tile([128, ROI], dt.float32, tag=f"a{cv}p{pk}", name=f"a{cv}p{pk}")
                   for pk in range(PACKS)] for cv in range(2)]
            u3row = ppool.tile([1, NLOC], dt.float32, tag="u3row", name="u3row")
            v3row = ppool.tile([1, NLOC], dt.float32, tag="v3row", name="v3row")
            A3all = ppool.tile([128, 24], dt.float32, tag="A3all", name="A3all")
            t3scr = ppool.tile([128, ROI], dt.float32, tag="t3scr", name="t3scr")
            packR = ppool.tile([96, ROI], dt.float32, tag="packR", name="packR")
            wrapped = [ppool.tile([16, EG // 16], dt.int16, tag=f"wr{g}", name=f"wr{g}")
                       for g in range(GPC)]
            stats6 = ppool.tile([128, 10], dt.float32, tag="stats6", name="stats6")
            nc.vector.memset(A3all[:], 0.0)
            sq_scratch = ppool.tile([128, ROI], dt.float32, tag="sq_scratch", name="sq_scratch")

            # ================================================================
            # Stage 1: per graph-pair: load xT, squares, projections, d2 + topk
            # ================================================================
            def topk32(keyS, csz, gl, ic):
                """keyS [csz<=128, ROI] f32 SBUF (destroyed). Writes wrapped[gl]
                columns for i-chunk ic (ic in 0,1) or returns idxf for packR."""
                idxu = spool.tile([128, K], dt.uint32, tag="idxu", name="idxu")
                for r in range(4):
                    m8 = spool.tile([128, 8], dt.float32, tag=f"m8_{r % 2}", name=f"m8_{r % 2}")
                    nc.vector.max(m8[:csz, :], keyS[:csz, :])
                    nc.vector.max_index(idxu[:csz, 8 * r:8 * r + 8], m8[:csz, :],
                                        keyS[:csz, :])
                    if r < 3:
                        nc.vector.match_replace(keyS[:csz, :], m8[:csz, :],
                                                keyS[:csz, :], -1e30)
                idxf = spool.tile([128, K], dt.float32, tag="idxf", name="idxf")
                nc.vector.tensor_copy(idxf[:csz, :], idxu[:csz, :])
                return idxf

            def idx_to_wrapped(idxf, csz, dst_list):
                """PE-transpose idxf [csz, 32] halves; dst_list = list of
                (wrapped_tile, col_slice_for_even, col_slice_for_odd, src_cols)"""
                pT0 = psB.tile([16, 128], dt.float32, tag="B", name="pT0")
                pT1 = psB.tile([16, 128], dt.float32, tag="B", name="pT1")
                nc.tensor.transpose(pT0[:, :csz], idxf[:csz, 0:16],
                                    ident[:csz, :csz])
                nc.tensor.transpose(pT1[:, :csz], idxf[:csz, 16:32],
                                    ident[:csz, :csz])
                for wr, ev, od, (c0, cn) in dst_list:
                    w2 = wr[:].rearrange("p (i two) -> p i two", two=2)
                    nc.scalar.copy(w2[:, ev[0]:ev[0] + ev[1], 0],
                                   pT0[:, c0:c0 + cn])
                    nc.scalar.copy(w2[:, od[0]:od[0] + od[1], 1],
                                   pT1[:, c0:c0 + cn])

            def pair_stage(pr):
                xt = [xpool.tile([sz, 2 * ROI], dt.float32, tag=f"xt{i}", name=f"xt{i}")
                      for i, (o, sz) in enumerate(FCH)]
                for i, (o, sz) in enumerate(FCH):
                    nc.sync.dma_start(
                        xt[i][:], d_xT.ap()[o:o + sz,
                                            2 * ROI * pr:2 * ROI * (pr + 1)])
                # squared features + (-0.5) * column sums -> nsqrow
                sqt = [xpool.tile([sz, 2 * ROI], dt.float32, tag=f"sqt{i}", name=f"sqt{i}")
                       for i, (o, sz) in enumerate(FCH)]
                for i in range(3):
                    nc.scalar.square(sqt[i][:], xt[i][:])
                nsqrow = spool.tile([1, 2 * ROI], dt.float32, tag="nsqrow", name="nsqrow")
                for h in range(2):
                    pnsq = psB.tile([1, ROI], dt.float32, tag="B", name="pnsq")
                    for i, (o, sz) in enumerate(FCH):
                        nc.tensor.matmul(pnsq[:], ones_col[:sz, :],
                                         sqt[i][:, ROI * h:ROI * (h + 1)],
                                         start=(i == 0), stop=(i == 2))
                    nc.scalar.activation(nsqrow[:, ROI * h:ROI * (h + 1)],
                                         pnsq[:], AF.Copy, scale=-0.5)

                for h in range(2):              # graphs gl = 2*pr + h
                    gl = 2 * pr + h
                    pk, q = gl // 4, gl % 4
                    # ---- projections [u1|v1|cc2|v2] ----
                    pproj = psA.tile([128, ROI], dt.float32, tag="A", name="pproj")
                    for i, (o, sz) in enumerate(FCH):
                        nc.tensor.matmul(pproj[:], wproj[i][:],
                                         xt[i][:, ROI * h:ROI * (h + 1)],
                                         start=(i == 0), stop=(i == 2))
                    for cv in range(2):
                        nc.scalar.activation(
                            Up[cv][pk][32 * q:32 * q + 32, :],
                            pproj[64 * cv:64 * cv + 32, :], AF.Identity,
                            bias=bpack[32 * q:32 * q + 32, cv:cv + 1])
                        nc.scalar.copy(Vp[cv][pk][32 * q:32 * q + 32, :],
                                       pproj[64 * cv + 32:64 * cv + 64, :])
                    # ---- u3/v3 ----
                    pproj3a = psB.tile([1, ROI], dt.float32, tag="B", name="pproj3a")
                    pproj3b = psB.tile([1, ROI], dt.float32, tag="B", name="pproj3b")
                    for i, (o, sz) in enumerate(FCH):
                        nc.tensor.matmul(pproj3a[:], wproj3[i][:, 0:1],
                                         xt[i][:, ROI * h:ROI * (h + 1)],
                                         start=(i == 0), stop=(i == 2))
                    for i, (o, sz) in enumerate(FCH):
                        nc.tensor.matmul(pproj3b[:], wproj3[i][:, 1:2],
                                         xt[i][:, ROI * h:ROI * (h + 1)],
                                         start=(i == 0), stop=(i == 2))
                    nc.scalar.copy(u3row[:, ROI * gl:ROI * (gl + 1)], pproj3a[:])
                    nc.scalar.copy(v3row[:, ROI * gl:ROI * (gl + 1)], pproj3b[:])

                for h in range(2):              # d2/topk after both projections
                    gl = 2 * pr + h
                    # ---- d2 key + top-32 per i-chunk ----
                    for ic, (io, isz) in enumerate([(0, 128), (128, 128),
                                                    (256, 12)]):
                        pkey = psA.tile([128, ROI], dt.float32, tag="A", name="pkey")
                        for i, (o, sz) in enumerate(FCH):
                            nc.tensor.matmul(
                                pkey[:isz, :],
                                xt[i][:, ROI * h + io:ROI * h + io + isz],
                                xt[i][:, ROI * h:ROI * (h + 1)],
                                start=(i == 0), stop=False)
                        nc.tensor.matmul(pkey[:isz, :], ones_row[:, :isz],
                                         nsqrow[:, ROI * h:ROI * (h + 1)],
                                         start=False, stop=True)
                        if ic < 2:
                            keyS = spool.tile([128, ROI], dt.float32, tag="keyS", name="keyS")
                            nc.scalar.copy(keyS[:], pkey[:])
                            if taps and gl == 0 and ic == 0:
                                nc.sync.dma_start(d_taps["tap_key0"].ap(), keyS[:])
                            idxf = topk32(keyS, 128, gl, ic)
                            idx_to_wrapped(
                                idxf, 128,
                                [(wrapped[gl], (128 * ic, 128), (128 * ic, 128),
                                  (0, 128))])
                        else:
                            rstage = spool.tile([12, ROI], dt.float32,
                                                tag="rstage", name="rstage")
                            nc.scalar.copy(rstage[:], pkey[:12, :])
                            nc.sync.dma_start(packR[12 * gl:12 * gl + 12, :],
                                              rstage[:])

            idx2sb = [ppool.tile([128, EG // 16], dt.int16, tag=f"idx2_{t}", name=f"idx2_{t}")
                      for t in range(PACKS)]

            # remainder rows topk ([96, ROI] packed, 12 rows per graph)
            def do_packR_topk():
              idxfR = topk32(packR, 96, -1, -1)
              pTR0 = psB.tile([16, 96], dt.float32, tag="B", name="pTR0")
              pTR1 = psB.tile([16, 96], dt.float32, tag="B", name="pTR1")
              nc.tensor.transpose(pTR0[:], idxfR[:96, 0:16], ident[:96, :96])
              nc.tensor.transpose(pTR1[:], idxfR[:96, 16:32], ident[:96, :96])
              for g in range(GPC):
                  w2 = wrapped[g][:].rearrange("p (i two) -> p i two", two=2)
                  nc.scalar.copy(w2[:, 256:268, 0], pTR0[:, 12 * g:12 * g + 12])
                  nc.scalar.copy(w2[:, 256:268, 1], pTR1[:, 12 * g:12 * g + 12])
              if taps:
                  wr0f = spool.tile([16, EG // 16], dt.float32, tag="wr0f", name="wr0f")
                  nc.vector.tensor_copy(wr0f[:], wrapped[0][:])
                  nc.sync.dma_start(d_taps["tap_wr0"].ap(), wr0f[:])

              # device-built gcn2 gather index packs
              for g in range(GPC):
                  pk, q = g // 4, g % 4
                  nc.sync.dma_start(idx2sb[pk][32 * q:32 * q + 16, :], wrapped[g][:])
                  nc.sync.dma_start(idx2sb[pk][32 * q + 16:32 * q + 32, :],
                                    wrapped[g][:])

            # ================================================================
            # Stage 3: edge stages (gcn1, gcn2) + gcn3
            # ================================================================
            def edge_gather(cv, pk):
                idxp = idx1sb if cv == 0 else idx2sb
                Gv = epool.tile([128, EG], dt.float32, tag="Gv", name="Gv")
                nc.gpsimd.ap_gather(Gv[:], Vp[cv][pk][:], idxp[pk][:],
                                    channels=128, num_elems=ROI, d=1,
                                    num_idxs=EG)
                return Gv

            def edge_compute(cv, pk, Gv):
                g3 = Gv[:].rearrange("p (i k) -> p i k", k=DEG)
                ub = Up[cv][pk][:].unsqueeze(2).broadcast_to([128, ROI, DEG])
                nc.vector.tensor_tensor(g3, g3, ub, op=OP.add)
                Gb = epool.tile([128, EG], dt.bfloat16, tag="Gb", name="Gb")
                nc.scalar.activation(Gb[:], Gv[:], AF.Prelu, alpha=SLOPE)
                for ec, (eo, en) in enumerate(ECH):
                    pm = psM.tile([128, 512], dt.float32, tag="M", name="pm")
                    nc.tensor.matmul(pm[:, :en], wbd[cv][:],
                                     Gb[:, eo:eo + en], start=True, stop=True)
                    nc.vector.tensor_reduce(
                        aT[cv][pk][:, eo // DEG:(eo + en) // DEG],
                        pm[:, :en].rearrange("p (i k) -> p i k", k=DEG),
                        axis=AX.X, op=OP.max)
                sc = 4 * cv + pk
                qc = 4 * cv + 2 + pk
                nc.vector.tensor_reduce(stats6[:, sc:sc + 1],
                                        aT[cv][pk][:], axis=AX.X, op=OP.add)
                nc.scalar.activation(
                    sq_scratch[:], aT[cv][pk][:], AF.Square,
                    accum_out=stats6[:, qc:qc + 1])

            # CC-stream warm-up: dummy AllReduce on scratch, hidden under stage 1
            warm_in = dpool.tile([33, 6], dt.float32, name="warm_in")
            warm_out = dpool.tile([33, 6], dt.float32, name="warm_out")
            warmsrc = spool.tile([33, 6], dt.float32, tag="warmsrc", name="warmsrc")
            nc.vector.memset(warmsrc[:], 0.0)
            nc.sync.dma_start(warm_in[:], warmsrc[:])
            if not single:
                nc.gpsimd.collective_compute(
                    "AllReduce", OP.add, replica_groups=[list(range(NCORES))],
                    ins=[warm_in[:]], outs=[warm_out[:]])

            pair_stage(0)
            pair_stage(1)
            gv00 = edge_gather(0, 0)
            pair_stage(2)
            pair_stage(3)
            gv01 = edge_gather(0, 1)
            do_packR_topk()
            edge_compute(0, 0, gv00)
            gv10 = edge_gather(1, 0)
            edge_compute(0, 1, gv01)
            gv11 = edge_gather(1, 1)
            edge_compute(1, 0, gv10)

            # ---- gcn3: dense masked max (no gather) ----
            pA3u = psB.tile([128, 24], dt.float32, tag="B", name="pA3u")
            for g in range(GPC):
                pv3f = psA.tile([128, ROI], dt.float32, tag="A", name="pv3f")
                nc.tensor.matmul(pv3f[:],
                                 ones_row[0:1, :],
                                 v3row[0:1, ROI * g:ROI * (g + 1)],
                                 start=True, stop=True)
                for c, csz in enumerate((128, 128, 12)):
                    col = 3 * g + c
                    nc.vector.tensor_tensor(
                        t3scr[0:csz, :],
                        mask3[0:csz, ROI * col:ROI * (col + 1)],
                        pv3f[0:csz, :], op=OP.add)
                    nc.vector.tensor_reduce(
                        A3all[0:csz, col:col + 1], t3scr[0:csz, :],
                        axis=AX.X, op=OP.max)
                    nc.tensor.matmul(
                        pA3u[0:csz, col:col + 1],
                        u3row[0:1, ROI * g + 128 * c:ROI * g + 128 * c + csz],
                        ones_row[0:1, 0:1],
                        start=True, stop=True)
            for g in range(GPC):
                for c, csz in enumerate((128, 128, 12)):
                    col = 3 * g + c
                    nc.vector.tensor_tensor(A3all[0:csz, col:col + 1],
                                            A3all[0:csz, col:col + 1],
                                            pA3u[0:csz, col:col + 1], op=OP.add)
            nc.vector.tensor_reduce(stats6[:, 8:9], A3all[:], axis=AX.X, op=OP.add)
            sq24 = spool.tile([128, 24], dt.float32, tag="sq24", name="sq24")
            nc.scalar.activation(sq24[:], A3all[:], AF.Square,
                                 accum_out=stats6[:, 9:10])

            edge_compute(1, 1, gv11)

            # fold partial stats into columns [s1 q1 s2 q2 s3 q3]
            stats_o = spool.tile([128, 6], dt.float32, tag="stats_o", name="stats_o")
            nc.vector.tensor_tensor(stats_o[:, 0:1], stats6[:, 0:1],
                                    stats6[:, 1:2], op=OP.add)
            nc.vector.tensor_tensor(stats_o[:, 1:2], stats6[:, 2:3],
                                    stats6[:, 3:4], op=OP.add)
            nc.vector.tensor_tensor(stats_o[:, 2:3], stats6[:, 4:5],
                                    stats6[:, 5:6], op=OP.add)
            nc.vector.tensor_tensor(stats_o[:, 3:4], stats6[:, 6:7],
                                    stats6[:, 7:8], op=OP.add)
            nc.vector.tensor_copy(stats_o[:, 4:5], stats6[:, 8:9])
            nc.vector.tensor_copy(stats_o[:, 5:6], stats6[:, 9:10])
            pfold = psB.tile([33, 6], dt.float32, tag="B", name="pfold")
            nc.tensor.matmul(pfold[:], foldM[:], stats_o[:], start=True, stop=True)
            statsloc = spool.tile([33, 6], dt.float32, tag="statsloc", name="statsloc")
            nc.scalar.copy(statsloc[:], pfold[:])

            # ================================================================
            # Stage 4: AllReduce stats; BN affine params; x-stage; pooling
            # ================================================================
            b_in = dpool.tile([33, 6], dt.float32)
            b_out = dpool.tile([33, 6], dt.float32)
            nc.sync.dma_start(b_in[:], statsloc[:])
            if single:
                nc.sync.dma_start(b_out[:], b_in[:])
            else:
                nc.gpsimd.collective_compute(
                    "AllReduce", OP.add, replica_groups=[list(range(NCORES))],
                    ins=[b_in[:]], outs=[b_out[:]])
            statsg = spool.tile([33, 6], dt.float32, tag="statsg", name="statsg")
            nc.sync.dma_start(statsg[:], b_out[:])
            if taps:
                nc.sync.dma_start(d_taps["tap_stats"].ap(), statsg[:])
            # move conv3 stats (cols 4,5 at partition 32) into cols 0,1
            nc.scalar.copy(statsg[32:33, 0:2], statsg[32:33, 4:6])
            scaled = spool.tile([33, 4], dt.float32, tag="scaled", name="scaled")
            nc.vector.tensor_scalar_mul(scaled[:], statsg[:, 0:4], 1.0 / NTOT)
            var = spool.tile([33, 2], dt.float32, tag="var", name="var")
            sA = spool.tile([33, 4], dt.float32, tag="sA", name="sA")  # [sA tA sB tB]
            mu = scaled[:].rearrange("p (c two) -> p c two", two=2)
            nc.vector.tensor_tensor(var[:], mu[:, :, 0], mu[:, :, 0], op=OP.mult)
            nc.vector.tensor_tensor(var[:], mu[:, :, 1], var[:], op=OP.subtract)
            nc.vector.tensor_scalar_max(var[:], var[:], 0.0)
            nc.vector.tensor_scalar_add(var[:], var[:], EPS)
            nc.scalar.sqrt(var[:], var[:])
            nc.vector.reciprocal(var[:], var[:])
            sA2 = sA[:].rearrange("p (c two) -> p c two", two=2)
            gbe2 = gbe[:].rearrange("p (c two) -> p c two", two=2)
            nc.vector.tensor_tensor(sA2[:, :, 0], gbe2[:, :, 0], var[:], op=OP.mult)
            nc.vector.tensor_tensor(sA2[:, :, 1], mu[:, :, 0], sA2[:, :, 0],
                                    op=OP.mult)
            nc.vector.tensor_tensor(sA2[:, :, 1], gbe2[:, :, 1], sA2[:, :, 1],
                                    op=OP.subtract)
            paff1 = psB.tile([128, 4], dt.float32, tag="B", name="paff1")
            paff3 = psB.tile([128, 2], dt.float32, tag="B", name="paff3")
            nc.tensor.matmul(paff1[:], selM[:, 0:128], sA[:], start=True, stop=True)
            nc.tensor.matmul(paff3[:], selM[:, 128:256], sA[:, 0:2],
                             start=True, stop=True)
            aff1 = spool.tile([128, 4], dt.float32, tag="aff1", name="aff1")
            aff3 = spool.tile([128, 2], dt.float32, tag="aff3", name="aff3")
            nc.scalar.copy(aff1[:], paff1[:])
            nc.scalar.copy(aff3[:], paff3[:])
            if taps:
                nc.sync.dma_start(d_taps["tap_aff1"].ap(), aff1[:])
                for cv in range(2):
                    for pk in range(PACKS):
                        nc.sync.dma_start(
                            d_taps[f"tap_a{cv + 1}T{pk}"].ap(), aT[cv][pk][:])
                nc.sync.dma_start(d_taps["tap_a3row"].ap(), a3row[:])

            pool4 = spool.tile([128, 4], dt.float32, tag="pool4", name="pool4")
            for cv in range(2):
                for pk in range(PACKS):
                    nc.scalar.activation(aT[cv][pk][:], aT[cv][pk][:],
                                         AF.Prelu, alpha=SLOPE,
                                         scale=aff1[:, 2 * cv:2 * cv + 1],
                                         bias=aff1[:, 2 * cv + 1:2 * cv + 2])
                    nc.vector.tensor_reduce(pool4[:, 2 * cv + pk:2 * cv + pk + 1],
                                            aT[cv][pk][:], axis=AX.X, op=OP.add)
            A3f = spool.tile([128, 24], dt.float32, tag="A3f", name="A3f")
            nc.scalar.activation(A3f[:], A3all[:], AF.Prelu, alpha=SLOPE,
                                 scale=aff3[:, 0:1], bias=aff3[:, 1:2])

            # ================================================================
            # Stage 5: assemble zT [332, 8], AllGather -> [332, 64]
            # ================================================================
            zT0 = spool.tile([64, GPC], dt.float32, tag="zT0", name="zT0")
            g_in = dpool.tile([332, GPC], dt.float32, name="g_in")
            for g in range(GPC):
                pk, q = g // 4, g % 4
                nc.sync.dma_start(zT0[0:32, g:g + 1],
                                  pool4[32 * q:32 * q + 32, pk:pk + 1])
                nc.sync.dma_start(zT0[32:64, g:g + 1],
                                  pool4[32 * q:32 * q + 32, 2 + pk:3 + pk])
                nc.sync.dma_start(g_in[64:192, g:g + 1],
                                  A3f[:, 3 * g:3 * g + 1])
                nc.sync.dma_start(g_in[192:320, g:g + 1],
                                  A3f[:, 3 * g + 1:3 * g + 2])
                nc.sync.dma_start(g_in[320:332, g:g + 1],
                                  A3f[0:12, 3 * g + 2:3 * g + 3])
            g_out = dpool.tile([NCORES, 332, GPC], dt.float32, name="g_out")
            nc.sync.dma_start(g_in[0:64, :], zT0[:])
            if single:
                for cc in range(NCORES):
                    nc.sync.dma_start(g_out[cc, :, :], g_in[:])
            else:
                nc.gpsimd.collective_compute(
                    "AllGather", OP.bypass, replica_groups=[list(range(NCORES))],
                    ins=[g_in[:]], outs=[g_out[:]])
            zAll = [spool.tile([sz, B], dt.float32, tag=f"zAll{i}", name=f"zAll{i}")
                    for i, (o, sz) in enumerate(MCH)]
            for i, (o, sz) in enumerate(MCH):
                nc.sync.dma_start(
                    zAll[i][:].rearrange("r (c j) -> r c j", j=GPC),
                    g_out[:, o:o + sz, :].transpose([1, 0, 2]))
            if taps:
                nc.sync.dma_start(d_taps["tap_za"].ap(), zAll[0][:])
                nc.sync.dma_start(d_taps["tap_zb"].ap(), zAll[1][:])
                nc.sync.dma_start(d_taps["tap_zc"].ap(), zAll[2][:])

            # ================================================================
            # Stage 6: readout MLP, replicated on every core
            # ================================================================
            def bn_affine_cols(pml, gbe_t, col0, tag):
                """pml psum [128, B]; returns (s, t) [128,1] tiles."""
                s_sum = spool.tile([128, 1], dt.float32, tag=f"{tag}_sum", name=f"{tag}_sum")
                q_sum = spool.tile([128, 1], dt.float32, tag=f"{tag}_q", name=f"{tag}_q")
                scr = spool.tile([128, B], dt.float32, tag="mlp_scr", name="mlp_scr")
                nc.vector.tensor_reduce(s_sum[:], pml[:], axis=AX.X, op=OP.add)
                nc.scalar.activation(scr[:], pml[:], AF.Square, accum_out=q_sum[:])
                nc.vector.tensor_scalar_mul(s_sum[:], s_sum[:], 1.0 / B)
                nc.vector.tensor_scalar_mul(q_sum[:], q_sum[:], 1.0 / B)
                v = spool.tile([128, 1], dt.float32, tag=f"{tag}_v", name=f"{tag}_v")
                nc.vector.tensor_tensor(v[:], s_sum[:], s_sum[:], op=OP.mult)
                nc.vector.tensor_tensor(v[:], q_sum[:], v[:], op=OP.subtract)
                nc.vector.tensor_scalar_add(v[:], v[:], EPS)
                nc.scalar.sqrt(v[:], v[:])
                nc.vector.reciprocal(v[:], v[:])
                s_t = spool.tile([128, 1], dt.float32, tag=f"{tag}_s", name=f"{tag}_s")
                t_t = spool.tile([128, 1], dt.float32, tag=f"{tag}_t", name=f"{tag}_t")
                nc.vector.tensor_tensor(s_t[:], gbe_t[:, col0:col0 + 1], v[:],
                                        op=OP.mult)
                nc.vector.tensor_tensor(t_t[:], s_sum[:], s_t[:], op=OP.mult)
                nc.vector.tensor_tensor(t_t[:], gbe_t[:, col0 + 1:col0 + 2],
                                        t_t[:], op=OP.subtract)
                return s_t, t_t

            z1T = [spool.tile([128, B], dt.float32, tag=f"z1T{m}", name=f"z1T{m}")
                   for m in range(2)]
            for m in range(2):
                ph1 = psM.tile([128, B], dt.float32, tag="M", name="ph1")
                for i, (o, sz) in enumerate(MCH):
                    nc.tensor.matmul(ph1[:], wl1[i][:, 128 * m:128 * m + 128],
                                     zAll[i][:], start=(i == 0), stop=(i == 2))
                s_t, t_t = bn_affine_cols(ph1[:], gbe4, 2 * m, f"bn4_{m}")
                nc.scalar.activation(z1T[m][:], ph1[:], AF.Prelu, alpha=SLOPE,
                                     scale=s_t[:], bias=t_t[:])
            ph2 = psM.tile([128, B], dt.float32, tag="M", name="ph2")
            for m in range(2):
                nc.tensor.matmul(ph2[:], wl2[m][:], z1T[m][:],
                                 start=(m == 0), stop=(m == 1))
            s_t, t_t = bn_affine_cols(ph2[:], gbe5, 0, "bn5")
            z2T = spool.tile([128, B], dt.float32, tag="z2T", name="z2T")
            nc.scalar.activation(z2T[:], ph2[:], AF.Prelu, alpha=SLOPE,
                                 scale=s_t[:], bias=t_t[:])
            pout = psM.tile([1, B], dt.float32, tag="M", name="pout")
            nc.tensor.matmul(pout[:], wl3[:], z2T[:], start=True, stop=True)
            out_sb = spool.tile([1, B], dt.float32, tag="out_sb", name="out_sb")
            nc.scalar.activation(out_sb[:], pout[:], AF.Identity, bias=bl3[:])
            nc.sync.dma_start(d_out.ap(), out_sb[:])

    nc.compile()
    return nc


def _host_prep(inputs):
    x = np.asarray(inputs["x"])
    ei = np.asarray(inputs["edge_index"])
    src = ei[0]
    tgt = ei[1]
    exp_tgt = np.repeat(np.arange(NTOT, dtype=np.int64), DEG)
    assert np.array_equal(tgt.astype(np.int64), exp_tgt), \
        "edge_index structure mismatch (expected DGCN static grid)"
    assert np.array_equal(src // ROI, tgt // ROI), "cross-graph edges found"
    src_local = (src % ROI).astype(np.int16).reshape(B, ROI, DEG)

    # wrapped edge lists: el [B, EG]; wr [B, 16, EG//16]
    el = src_local.reshape(B, EG)
    wr = el.reshape(B, EG // 16, 16).transpose(0, 2, 1)  # pos n=(col*16+p)

    W1a, W2a, W3 = (_fp32(inputs[k]) for k in ("W1a", "W2a", "W3"))
    A1, B1 = W1a[:F] - W1a[F:], W1a[F:]
    A2, B2 = W2a[:F] - W2a[F:], W2a[F:]
    A3, B3 = W3[:F] - W3[F:], W3[F:]

    wproj = _fp32(np.concatenate([A1, B1, A2, B2], axis=1))
    wproj3 = _fp32(np.concatenate([A3, B3], axis=1))

    import ml_dtypes

    def blockdiag(w):
        out = np.zeros((128, 128), np.float32)
        for q in range(4):
            out[32 * q:32 * q + 32, 32 * q:32 * q + 32] = w
        return out

    wbd1 = blockdiag(_fp32(inputs["W1b"])).astype(ml_dtypes.bfloat16)
    wbd2 = blockdiag(_fp32(inputs["W2b"])).astype(ml_dtypes.bfloat16)
    bpack = _fp32(np.stack([np.tile(inputs["b1a"], 4),
                            np.tile(inputs["b2a"], 4)], axis=1))
    ident = np.eye(128, dtype=np.float32)

    foldM = np.zeros((128, 33), np.float32)
    p = np.arange(128)
    foldM[p, p % 32] = 1.0
    foldM[:, 32] = 1.0
    sel32 = np.zeros((33, 128), np.float32)
    sel32[p % 32, p] = 1.0
    sel3 = np.zeros((33, 128), np.float32)
    sel3[32, :] = 1.0
    selM = _fp32(np.concatenate([sel32, sel3], axis=1))

    gbe = np.zeros((33, 4), np.float32)
    gbe[0:32, 0] = inputs["g1"]
    gbe[0:32, 1] = inputs["be1"]
    gbe[0:32, 2] = inputs["g2"]
    gbe[0:32, 3] = inputs["be2"]
    gbe[32, 0] = inputs["g3"][0]
    gbe[32, 1] = inputs["be3"][0]


    wl1 = _fp32(inputs["Wl1"]).copy()
    wl1[0:64, :] /= ROI          # mean pooling folded into the weights
    wl2 = _fp32(inputs["Wl2"])
    wl3 = _fp32(inputs["Wl3"])
    gbe4 = _fp32(np.stack([inputs["g4"][0:128], inputs["be4"][0:128],
                           inputs["g4"][128:256], inputs["be4"][128:256]],
                          axis=1))
    gbe5 = _fp32(np.stack([inputs["g5"], inputs["be5"]], axis=1))
    bl3 = _fp32(inputs["bl3"].reshape(1, 1))

    shared = dict(ident=ident, wproj=wproj, wproj3=wproj3, wbd1=wbd1,
                  wbd2=wbd2, bpack=bpack, foldM=foldM, selM=selM, gbe=gbe,
                  wl1=wl1, wl2=wl2, wl3=wl3, gbe4=gbe4,
                  gbe5=gbe5, bl3=bl3)

    in_maps = []
    for c in range(NCORES):
        gs = slice(GPC * c, GPC * (c + 1))
        xT = _fp32(x[NLOC * c:NLOC * (c + 1)].T)
        idx1 = np.empty((2 * 128, EG // 16), np.int16)
        for t in range(2):
            for qq in range(4):
                g = GPC * c + 4 * t + qq
                idx1[128 * t + 32 * qq:128 * t + 32 * qq + 16] = wr[g]
                idx1[128 * t + 32 * qq + 16:128 * t + 32 * qq + 32] = wr[g]
        mask3 = np.full((128, 24, ROI), -1e30, np.float32)
        for g in range(GPC):
            sl = el[GPC * c + g].reshape(ROI, DEG)
            for cc3, csz3 in enumerate((128, 128, 12)):
                for pp in range(csz3):
                    mask3[pp, 3 * g + cc3, sl[128 * cc3 + pp]] = 0.0
        mask3 = mask3.reshape(128, 24 * ROI).astype(ml_dtypes.bfloat16)
        m = dict(shared)
        m.update(xT=xT, idx1=np.ascontiguousarray(idx1),
                 mask3=np.ascontiguousarray(mask3))
        in_maps.append(m)
    return in_maps


def kernel(**inputs):
    from concourse.bass_utils import run_bass_kernel_spmd

    if "nc" not in _cache:
        _cache["nc"] = _build_program()
    nc = _cache["nc"]
    in_maps = _host_prep(inputs)
    trace = bool(int(os.environ.get("KERNEL_TRACE", "0")))
    tmpdir = os.environ.get("KERNEL_TMPDIR") or None
    if tmpdir:
        os.makedirs(tmpdir, exist_ok=True)
    res = run_bass_kernel_spmd(nc, in_maps, core_ids=list(range(NCORES)),
                               trace=trace, tmpdir=tmpdir)
    _cache["last_results"] = res
    out = res.results[0]["out"].reshape(B, 1)
    return np.ascontiguousarray(out, dtype=np.float32)



# revision 6
# speedup vs baseline: 1.0747x; 1.0335x over previous
"""DGCN (EdgeConv x2 + DynamicEdgeConv + readout MLP) on 8 TRN2 NeuronCores.

Sharding: graph-level data parallel. 64 graphs -> 8 cores x 8 graphs.
Within a core, graphs are processed as 2 "packs" of 4 graphs (4 x 32ch = 128
partitions). All activations live transposed (channels/features on the
partition axis, nodes/edges on the free axis) so that:
  - projections/Gram matrices are plain matmuls over the feature axis,
  - EdgeConv neighbor gathers are single GPSIMD ap_gather ops along the
    free axis (per-16-partition-group index lists),
  - per-edge MLPs are block-diagonal 128-contraction matmuls,
  - BatchNorm affine+LeakyReLU collapse into one ScalarE activation.
BatchNorm statistics are global over all 17152 nodes -> one tiny AllReduce;
the readout MLP (BN over the 64-graph batch) runs replicated on every core
after an AllGather of the 332-dim per-graph feature vectors.
"""

import os
import sys

sys.path.insert(0, "/opt/trn_rl_repo")

import numpy as np

B = 64
ROI = 268
F = 268
C = 32
K = 32
DEG = 32
NCORES = 8
GPC = B // NCORES          # graphs per core = 8
NLOC = GPC * ROI           # nodes per core = 2144
NTOT = B * ROI             # 17152
EG = ROI * DEG             # edges per graph = 8576
PACKS = 2                  # 4-graph packs per core
SLOPE = 0.33
EPS = 1e-5

_cache = {}


def _fp32(a):
    return np.ascontiguousarray(a, dtype=np.float32)


def _build_program():
    import concourse.bacc as bacc
    import concourse.tile as tile
    import concourse.mybir as mybir
    from concourse import bass

    dt = mybir.dt
    f32r = dt.float32r
    AF = mybir.ActivationFunctionType
    OP = mybir.AluOpType
    AX = mybir.AxisListType

    taps = False
    single = bool(int(os.environ.get("KERNEL_SINGLE", "0")))

    nc = bacc.Bacc("TRN2", target_bir_lowering=False, debug=False,
                   num_devices=1 if single else NCORES)

    # ---- DRAM I/O -------------------------------------------------------
    d_xT = nc.dram_tensor("xT", [F, NLOC], dt.float32, kind="ExternalInput")
    d_idx1 = nc.dram_tensor("idx1", [2 * 128, EG // 16], dt.int16, kind="ExternalInput")
    d_ident = nc.dram_tensor("ident", [128, 128], dt.float32, kind="ExternalInput")
    d_wproj = nc.dram_tensor("wproj", [F, 128], dt.float32, kind="ExternalInput")
    d_wproj3 = nc.dram_tensor("wproj3", [F, 2], dt.float32, kind="ExternalInput")
    d_wbd1 = nc.dram_tensor("wbd1", [128, 128], dt.bfloat16, kind="ExternalInput")
    d_wbd2 = nc.dram_tensor("wbd2", [128, 128], dt.bfloat16, kind="ExternalInput")
    d_bpack = nc.dram_tensor("bpack", [128, 2], dt.float32, kind="ExternalInput")
    d_fold = nc.dram_tensor("foldM", [128, 33], dt.float32, kind="ExternalInput")
    d_sel = nc.dram_tensor("selM", [33, 256], dt.float32, kind="ExternalInput")
    d_gbe = nc.dram_tensor("gbe", [33, 4], dt.float32, kind="ExternalInput")
    d_mask3 = nc.dram_tensor("mask3", [128, 24 * ROI], dt.bfloat16,
                             kind="ExternalInput")
    d_wl1 = nc.dram_tensor("wl1", [332, 256], dt.float32, kind="ExternalInput")
    d_wl2 = nc.dram_tensor("wl2", [256, 128], dt.float32, kind="ExternalInput")
    d_wl3 = nc.dram_tensor("wl3", [128, 1], dt.float32, kind="ExternalInput")
    d_gbe4 = nc.dram_tensor("gbe4", [128, 4], dt.float32, kind="ExternalInput")
    d_gbe5 = nc.dram_tensor("gbe5", [128, 2], dt.float32, kind="ExternalInput")
    d_bl3 = nc.dram_tensor("bl3", [1, 1], dt.float32, kind="ExternalInput")
    d_out = nc.dram_tensor("out", [1, B], dt.float32, kind="ExternalOutput")

    d_taps = {}
    if taps:
        for nm, shp in [("tap_a1T0", [128, ROI]), ("tap_a1T1", [128, ROI]),
                        ("tap_a2T0", [128, ROI]), ("tap_a2T1", [128, ROI]),
                        ("tap_a3row", [128, ROI]), ("tap_stats", [33, 6]),
                        ("tap_aff1", [128, 4]), ("tap_pool", [128, 4]),
                        ("tap_za", [128, 64]), ("tap_zb", [128, 64]), ("tap_zc", [76, 64]), ("tap_wr0", [16, EG // 16]),
                        ("tap_key0", [128, ROI])]:
            d_taps[nm] = nc.dram_tensor(nm, shp, dt.float32, kind="ExternalOutput")

    FCH = [(0, 128), (128, 128), (256, 12)]      # feature-axis chunks
    ECH = [(i * 512, 512) for i in range(16)] + [(16 * 512, EG - 16 * 512)]

    with tile.TileContext(nc) as tc:
        with tc.tile_pool(name="const", bufs=1) as wpool, \
             tc.tile_pool(name="persist", bufs=1) as ppool, \
             tc.tile_pool(name="xt", bufs=2) as xpool, \
             tc.tile_pool(name="edge", bufs=2) as epool, \
             tc.tile_pool(name="scratch", bufs=2) as spool, \
             tc.tile_pool(name="psA", bufs=2, space="PSUM") as psA, \
             tc.tile_pool(name="psB", bufs=2, space="PSUM") as psB, \
             tc.tile_pool(name="psM", bufs=3, space="PSUM") as psM, \
             tc.tile_pool(name="dram", bufs=1, space="DRAM") as dpool:

            # ---- constants to SBUF -------------------------------------
            def load(name, shape, dtype, src):
                t = wpool.tile(shape, dtype, tag=name)
                nc.sync.dma_start(t[:], src)
                return t

            ident = load("ident", [128, 128], dt.float32, d_ident.ap())
            wproj = [load(f"wproj{i}", [sz, 128], dt.float32,
                          d_wproj.ap()[o:o + sz, :]) for i, (o, sz) in enumerate(FCH)]
            wproj3 = [load(f"wproj3{i}", [sz, 2], dt.float32,
                           d_wproj3.ap()[o:o + sz, :]) for i, (o, sz) in enumerate(FCH)]
            wbd = [load("wbd1", [128, 128], dt.bfloat16, d_wbd1.ap()),
                   load("wbd2", [128, 128], dt.bfloat16, d_wbd2.ap())]
            bpack = load("bpack", [128, 2], dt.float32, d_bpack.ap())
            foldM = load("foldM", [128, 33], dt.float32, d_fold.ap())
            selM = load("selM", [33, 256], dt.float32, d_sel.ap())
            gbe = load("gbe", [33, 4], dt.float32, d_gbe.ap())
            mask3 = load("mask3", [128, 24 * ROI], dt.bfloat16, d_mask3.ap())
            MCH = [(0, 128), (128, 128), (256, 76)]   # 332 rows of wl1 / zT
            wl1 = [load(f"wl1_{i}", [sz, 256], dt.float32,
                        d_wl1.ap()[o:o + sz, :]) for i, (o, sz) in enumerate(MCH)]
            wl2 = [load(f"wl2_{i}", [128, 128], dt.float32,
                        d_wl2.ap()[128 * i:128 * i + 128, :]) for i in range(2)]
            wl3 = load("wl3", [128, 1], dt.float32, d_wl3.ap())
            gbe4 = load("gbe4", [128, 4], dt.float32, d_gbe4.ap())
            gbe5 = load("gbe5", [128, 2], dt.float32, d_gbe5.ap())
            bl3 = load("bl3", [1, 1], dt.float32, d_bl3.ap())
            idx1sb = [load(f"idx1_{t}", [128, EG // 16], dt.int16,
                           d_idx1.ap()[128 * t:128 * t + 128, :]) for t in range(2)]

            ones_col = wpool.tile([128, 1], dt.float32, tag="ones_col", name="ones_col")
            nc.vector.memset(ones_col[:], 1.0)
            ones_row = wpool.tile([1, 128], dt.float32, tag="ones_row", name="ones_row")
            nc.vector.memset(ones_row[:], 1.0)

            # ---- persistent per-core tensors ---------------------------
            Vp = [[ppool.tile([128, ROI], dt.float32, tag=f"V{cv}p{pk}", name=f"V{cv}p{pk}")
                   for pk in range(PACKS)] for cv in range(2)]
            Up = [[ppool.tile([128, ROI], dt.float32, tag=f"U{cv}p{pk}", name=f"U{cv}p{pk}")
                   for pk in range(PACKS)] for cv in range(2)]
            aT = [[ppool.tile([128, ROI], dt.float32, tag=f"a{cv}p{pk}", name=f"a{cv}p{pk}")
                   for pk in range(PACKS)] for cv in range(2)]
            u3row = ppool.tile([1, NLOC], dt.float32, tag="u3row", name="u3row")
            v3row = ppool.tile([1, NLOC], dt.float32, tag="v3row", name="v3row")
            A3all = ppool.tile([128, 24], dt.float32, tag="A3all", name="A3all")
            t3scr = ppool.tile([128, ROI], dt.float32, tag="t3scr", name="t3scr")
            packR = ppool.tile([96, ROI], dt.float32, tag="packR", name="packR")
            wrapped = [ppool.tile([16, EG // 16], dt.int16, tag=f"wr{g}", name=f"wr{g}")
                       for g in range(GPC)]
            stats6 = ppool.tile([128, 10], dt.float32, tag="stats6", name="stats6")
            nc.vector.memset(A3all[:], 0.0)
            sq_scratch = ppool.tile([128, ROI], dt.float32, tag="sq_scratch", name="sq_scratch")

            # ================================================================
            # Stage 1: per graph-pair: load xT, squares, projections, d2 + topk
            # ================================================================
            def topk32(keyS, csz, gl, ic):
                """keyS [csz<=128, ROI] f32 SBUF (destroyed). Writes wrapped[gl]
                columns for i-chunk ic (ic in 0,1) or returns idxf for packR."""
                idxu = spool.tile([128, K], dt.uint32, tag="idxu", name="idxu")
                for r in range(4):
                    m8 = spool.tile([128, 8], dt.float32, tag=f"m8_{r % 2}", name=f"m8_{r % 2}")
                    nc.vector.max(m8[:csz, :], keyS[:csz, :])
                    nc.vector.max_index(idxu[:csz, 8 * r:8 * r + 8], m8[:csz, :],
                                        keyS[:csz, :])
                    if r < 3:
                        nc.vector.match_replace(keyS[:csz, :], m8[:csz, :],
                                                keyS[:csz, :], -1e30)
                idxf = spool.tile([128, K], dt.float32, tag="idxf", name="idxf")
                nc.vector.tensor_copy(idxf[:csz, :], idxu[:csz, :])
                return idxf

            def idx_to_wrapped(idxf, csz, dst_list):
                """PE-transpose idxf [csz, 32] halves; dst_list = list of
                (wrapped_tile, col_slice_for_even, col_slice_for_odd, src_cols)"""
                pT0 = psB.tile([16, 128], dt.float32, tag="B", name="pT0")
                pT1 = psB.tile([16, 128], dt.float32, tag="B", name="pT1")
                nc.tensor.transpose(pT0[:, :csz], idxf[:csz, 0:16],
                                    ident[:csz, :csz])
                nc.tensor.transpose(pT1[:, :csz], idxf[:csz, 16:32],
                                    ident[:csz, :csz])
                for wr, ev, od, (c0, cn) in dst_list:
                    w2 = wr[:].rearrange("p (i two) -> p i two", two=2)
                    nc.scalar.copy(w2[:, ev[0]:ev[0] + ev[1], 0],
                                   pT0[:, c0:c0 + cn])
                    nc.scalar.copy(w2[:, od[0]:od[0] + od[1], 1],
                                   pT1[:, c0:c0 + cn])

            def pair_stage(pr):
                xt = [xpool.tile([sz, 2 * ROI], dt.float32, tag=f"xt{i}", name=f"xt{i}")
                      for i, (o, sz) in enumerate(FCH)]
                for i, (o, sz) in enumerate(FCH):
                    nc.sync.dma_start(
                        xt[i][:], d_xT.ap()[o:o + sz,
                                            2 * ROI * pr:2 * ROI * (pr + 1)])
                # squared features + (-0.5) * column sums -> nsqrow
                sqt = [xpool.tile([sz, 2 * ROI], dt.float32, tag=f"sqt{i}", name=f"sqt{i}")
                       for i, (o, sz) in enumerate(FCH)]
                for i in range(3):
                    nc.scalar.square(sqt[i][:], xt[i][:])
                nsqrow = spool.tile([1, 2 * ROI], dt.float32, tag="nsqrow", name="nsqrow")
                for h in range(2):
                    pnsq = psB.tile([1, ROI], dt.float32, tag="B", name="pnsq")
                    for i, (o, sz) in enumerate(FCH):
                        nc.tensor.matmul(pnsq[:], ones_col[:sz, :],
                                         sqt[i][:, ROI * h:ROI * (h + 1)],
                                         start=(i == 0), stop=(i == 2))
                    nc.scalar.activation(nsqrow[:, ROI * h:ROI * (h + 1)],
                                         pnsq[:], AF.Copy, scale=-0.5)

                for h in range(2):              # graphs gl = 2*pr + h
                    gl = 2 * pr + h
                    pk, q = gl // 4, gl % 4
                    # ---- projections [u1|v1|cc2|v2] ----
                    pproj = psA.tile([128, ROI], dt.float32, tag="A", name="pproj")
                    for i, (o, sz) in enumerate(FCH):
                        nc.tensor.matmul(pproj[:], wproj[i][:],
                                         xt[i][:, ROI * h:ROI * (h + 1)],
                                         start=(i == 0), stop=(i == 2))
                    for cv in range(2):
                        nc.scalar.activation(
                            Up[cv][pk][32 * q:32 * q + 32, :],
                            pproj[64 * cv:64 * cv + 32, :], AF.Identity,
                            bias=bpack[32 * q:32 * q + 32, cv:cv + 1])
                        nc.scalar.copy(Vp[cv][pk][32 * q:32 * q + 32, :],
                                       pproj[64 * cv + 32:64 * cv + 64, :])
                    # ---- u3/v3 ----
                    pproj3a = psB.tile([1, ROI], dt.float32, tag="B", name="pproj3a")
                    pproj3b = psB.tile([1, ROI], dt.float32, tag="B", name="pproj3b")
                    for i, (o, sz) in enumerate(FCH):
                        nc.tensor.matmul(pproj3a[:], wproj3[i][:, 0:1],
                                         xt[i][:, ROI * h:ROI * (h + 1)],
                                         start=(i == 0), stop=(i == 2))
                    for i, (o, sz) in enumerate(FCH):
                        nc.tensor.matmul(pproj3b[:], wproj3[i][:, 1:2],
                                         xt[i][:, ROI * h:ROI * (h + 1)],
                                         start=(i == 0), stop=(i == 2))
                    nc.scalar.copy(u3row[:, ROI * gl:ROI * (gl + 1)], pproj3a[:])
                    nc.scalar.copy(v3row[:, ROI * gl:ROI * (gl + 1)], pproj3b[:])

                for h in range(2):              # d2/topk after both projections
                    gl = 2 * pr + h
                    # ---- d2 key + top-32 per i-chunk ----
                    for ic, (io, isz) in enumerate([(0, 128), (128, 128),
                                                    (256, 12)]):
                        pkey = psA.tile([128, ROI], dt.float32, tag="A", name="pkey")
                        for i, (o, sz) in enumerate(FCH):
                            nc.tensor.matmul(
                                pkey[:isz, :],
                                xt[i][:, ROI * h + io:ROI * h + io + isz],
                                xt[i][:, ROI * h:ROI * (h + 1)],
                                start=(i == 0), stop=False)
                        nc.tensor.matmul(pkey[:isz, :], ones_row[:, :isz],
                                         nsqrow[:, ROI * h:ROI * (h + 1)],
                                         start=False, stop=True)
                        if ic < 2:
                            keyS = spool.tile([128, ROI], dt.float32, tag="keyS", name="keyS")
                            nc.scalar.copy(keyS[:], pkey[:])
                            if taps and gl == 0 and ic == 0:
                                nc.sync.dma_start(d_taps["tap_key0"].ap(), keyS[:])
                            idxf = topk32(keyS, 128, gl, ic)
                            idx_to_wrapped(
                                idxf, 128,
                                [(wrapped[gl], (128 * ic, 128), (128 * ic, 128),
                                  (0, 128))])
                        else:
                            rstage = spool.tile([12, ROI], dt.float32,
                                                tag="rstage", name="rstage")
                            nc.scalar.copy(rstage[:], pkey[:12, :])
                            nc.sync.dma_start(packR[12 * gl:12 * gl + 12, :],
                                              rstage[:])

            idx2sb = [ppool.tile([128, EG // 16], dt.int16, tag=f"idx2_{t}", name=f"idx2_{t}")
                      for t in range(PACKS)]

            # remainder rows topk ([96, ROI] packed, 12 rows per graph)
            def do_packR_topk():
              idxfR = topk32(packR, 96, -1, -1)
              pTR0 = psB.tile([16, 96], dt.float32, tag="B", name="pTR0")
              pTR1 = psB.tile([16, 96], dt.float32, tag="B", name="pTR1")
              nc.tensor.transpose(pTR0[:], idxfR[:96, 0:16], ident[:96, :96])
              nc.tensor.transpose(pTR1[:], idxfR[:96, 16:32], ident[:96, :96])
              for g in range(GPC):
                  w2 = wrapped[g][:].rearrange("p (i two) -> p i two", two=2)
                  nc.scalar.copy(w2[:, 256:268, 0], pTR0[:, 12 * g:12 * g + 12])
                  nc.scalar.copy(w2[:, 256:268, 1], pTR1[:, 12 * g:12 * g + 12])
              if taps:
                  wr0f = spool.tile([16, EG // 16], dt.float32, tag="wr0f", name="wr0f")
                  nc.vector.tensor_copy(wr0f[:], wrapped[0][:])
                  nc.sync.dma_start(d_taps["tap_wr0"].ap(), wr0f[:])

              # device-built gcn2 gather index packs
              for g in range(GPC):
                  pk, q = g // 4, g % 4
                  nc.sync.dma_start(idx2sb[pk][32 * q:32 * q + 16, :], wrapped[g][:])
                  nc.sync.dma_start(idx2sb[pk][32 * q + 16:32 * q + 32, :],
                                    wrapped[g][:])

            # ================================================================
            # Stage 3: edge stages (gcn1, gcn2) + gcn3
            # ================================================================
            def edge_gather(cv, pk):
                idxp = idx1sb if cv == 0 else idx2sb
                Gv = epool.tile([128, EG], dt.float32, tag="Gv", name="Gv")
                nc.gpsimd.ap_gather(Gv[:], Vp[cv][pk][:], idxp[pk][:],
                                    channels=128, num_elems=ROI, d=1,
                                    num_idxs=EG)
                return Gv

            def edge_compute(cv, pk, Gv, usrc=None):
                g3 = Gv[:].rearrange("p (i k) -> p i k", k=DEG)
                u_t = usrc if usrc is not None else Up[cv][pk]
                ub = u_t[:].unsqueeze(2).broadcast_to([128, ROI, DEG])
                nc.vector.tensor_tensor(g3, g3, ub, op=OP.add)
                Gb = epool.tile([128, EG], dt.bfloat16, tag="Gb", name="Gb")
                nc.scalar.activation(Gb[:], Gv[:], AF.Prelu, alpha=SLOPE)
                for ec, (eo, en) in enumerate(ECH):
                    pm = psM.tile([128, 512], dt.float32, tag="M", name="pm")
                    nc.tensor.matmul(pm[:, :en], wbd[cv][:],
                                     Gb[:, eo:eo + en], start=True, stop=True)
                    nc.vector.tensor_reduce(
                        aT[cv][pk][:, eo // DEG:(eo + en) // DEG],
                        pm[:, :en].rearrange("p (i k) -> p i k", k=DEG),
                        axis=AX.X, op=OP.max)
                sc = 4 * cv + pk
                qc = 4 * cv + 2 + pk
                nc.vector.tensor_reduce(stats6[:, sc:sc + 1],
                                        aT[cv][pk][:], axis=AX.X, op=OP.add)
                nc.scalar.activation(
                    sq_scratch[:], aT[cv][pk][:], AF.Square,
                    accum_out=stats6[:, qc:qc + 1])

            # CC-stream warm-up: dummy AllReduce on scratch, hidden under stage 1
            warm_in = dpool.tile([33, 6], dt.float32, name="warm_in")
            warm_out = dpool.tile([33, 6], dt.float32, name="warm_out")
            warmsrc = spool.tile([33, 6], dt.float32, tag="warmsrc", name="warmsrc")
            nc.vector.memset(warmsrc[:], 0.0)
            nc.sync.dma_start(warm_in[:], warmsrc[:])
            if not single:
                nc.gpsimd.collective_compute(
                    "AllReduce", OP.add, replica_groups=[list(range(NCORES))],
                    ins=[warm_in[:]], outs=[warm_out[:]])

            pair_stage(0)
            pair_stage(1)
            gv00 = edge_gather(0, 0)
            pair_stage(2)
            pair_stage(3)
            gv01 = edge_gather(0, 1)
            # anti-hoist: chain compute(0,0)'s U operand to pair-3 output so the
            # scheduler cannot queue its DVE/ACT ops ahead of pair 2/3 work
            zc = ppool.tile([128, 1], dt.float32, tag="zc", name="zc")
            nc.vector.tensor_scalar_mul(zc[:], Vp[0][1][:, 0:1], 0.0)
            nc.vector.tensor_scalar_add(sq_scratch[:], Up[0][0][:], zc[:])
            do_packR_topk()
            edge_compute(0, 0, gv00, usrc=sq_scratch)
            gv10 = edge_gather(1, 0)
            edge_compute(0, 1, gv01)
            gv11 = edge_gather(1, 1)
            edge_compute(1, 0, gv10)

            # ---- gcn3: dense masked max (no gather) ----
            pA3u = psB.tile([128, 24], dt.float32, tag="B", name="pA3u")
            for g in range(GPC):
                pv3f = psA.tile([128, ROI], dt.float32, tag="A", name="pv3f")
                nc.tensor.matmul(pv3f[:],
                                 ones_row[0:1, :],
                                 v3row[0:1, ROI * g:ROI * (g + 1)],
                                 start=True, stop=True)
                for c, csz in enumerate((128, 128, 12)):
                    col = 3 * g + c
                    nc.vector.tensor_tensor(
                        t3scr[0:csz, :],
                        mask3[0:csz, ROI * col:ROI * (col + 1)],
                        pv3f[0:csz, :], op=OP.add)
                    nc.vector.tensor_reduce(
                        A3all[0:csz, col:col + 1], t3scr[0:csz, :],
                        axis=AX.X, op=OP.max)
                    nc.tensor.matmul(
                        pA3u[0:csz, col:col + 1],
                        u3row[0:1, ROI * g + 128 * c:ROI * g + 128 * c + csz],
                        ones_row[0:1, 0:1],
                        start=True, stop=True)
            for g in range(GPC):
                for c, csz in enumerate((128, 128, 12)):
                    col = 3 * g + c
                    nc.vector.tensor_tensor(A3all[0:csz, col:col + 1],
                                            A3all[0:csz, col:col + 1],
                                            pA3u[0:csz, col:col + 1], op=OP.add)
            nc.vector.tensor_reduce(stats6[:, 8:9], A3all[:], axis=AX.X, op=OP.add)
            sq24 = spool.tile([128, 24], dt.float32, tag="sq24", name="sq24")
            nc.scalar.activation(sq24[:], A3all[:], AF.Square,
                                 accum_out=stats6[:, 9:10])

            edge_compute(1, 1, gv11)

            # fold partial stats into columns [s1 q1 s2 q2 s3 q3]
            stats_o = spool.tile([128, 6], dt.float32, tag="stats_o", name="stats_o")
            nc.vector.tensor_tensor(stats_o[:, 0:1], stats6[:, 0:1],
                                    stats6[:, 1:2], op=OP.add)
            nc.vector.tensor_tensor(stats_o[:, 1:2], stats6[:, 2:3],
                                    stats6[:, 3:4], op=OP.add)
            nc.vector.tensor_tensor(stats_o[:, 2:3], stats6[:, 4:5],
                                    stats6[:, 5:6], op=OP.add)
            nc.vector.tensor_tensor(stats_o[:, 3:4], stats6[:, 6:7],
                                    stats6[:, 7:8], op=OP.add)
            nc.vector.tensor_copy(stats_o[:, 4:5], stats6[:, 8:9])
            nc.vector.tensor_copy(stats_o[:, 5:6], stats6[:, 9:10])
            pfold = psB.tile([33, 6], dt.float32, tag="B", name="pfold")
            nc.tensor.matmul(pfold[:], foldM[:], stats_o[:], start=True, stop=True)
            statsloc = spool.tile([33, 6], dt.float32, tag="statsloc", name="statsloc")
            nc.scalar.copy(statsloc[:], pfold[:])

            # ================================================================
            # Stage 4: AllReduce stats; BN affine params; x-stage; pooling
            # ================================================================
            b_in = dpool.tile([33, 6], dt.float32)
            b_out = dpool.tile([33, 6], dt.float32)
            nc.sync.dma_start(b_in[:], statsloc[:])
            if single:
                nc.sync.dma_start(b_out[:], b_in[:])
            else:
                nc.gpsimd.collective_compute(
                    "AllReduce", OP.add, replica_groups=[list(range(NCORES))],
                    ins=[b_in[:]], outs=[b_out[:]])
            statsg = spool.tile([33, 6], dt.float32, tag="statsg", name="statsg")
            nc.sync.dma_start(statsg[:], b_out[:])
            if taps:
                nc.sync.dma_start(d_taps["tap_stats"].ap(), statsg[:])
            # move conv3 stats (cols 4,5 at partition 32) into cols 0,1
            nc.scalar.copy(statsg[32:33, 0:2], statsg[32:33, 4:6])
            scaled = spool.tile([33, 4], dt.float32, tag="scaled", name="scaled")
            nc.vector.tensor_scalar_mul(scaled[:], statsg[:, 0:4], 1.0 / NTOT)
            var = spool.tile([33, 2], dt.float32, tag="var", name="var")
            sA = spool.tile([33, 4], dt.float32, tag="sA", name="sA")  # [sA tA sB tB]
            mu = scaled[:].rearrange("p (c two) -> p c two", two=2)
            nc.vector.tensor_tensor(var[:], mu[:, :, 0], mu[:, :, 0], op=OP.mult)
            nc.vector.tensor_tensor(var[:], mu[:, :, 1], var[:], op=OP.subtract)
            nc.vector.tensor_scalar_max(var[:], var[:], 0.0)
            nc.vector.tensor_scalar_add(var[:], var[:], EPS)
            nc.scalar.sqrt(var[:], var[:])
            nc.vector.reciprocal(var[:], var[:])
            sA2 = sA[:].rearrange("p (c two) -> p c two", two=2)
            gbe2 = gbe[:].rearrange("p (c two) -> p c two", two=2)
            nc.vector.tensor_tensor(sA2[:, :, 0], gbe2[:, :, 0], var[:], op=OP.mult)
            nc.vector.tensor_tensor(sA2[:, :, 1], mu[:, :, 0], sA2[:, :, 0],
                                    op=OP.mult)
            nc.vector.tensor_tensor(sA2[:, :, 1], gbe2[:, :, 1], sA2[:, :, 1],
                                    op=OP.subtract)
            paff1 = psB.tile([128, 4], dt.float32, tag="B", name="paff1")
            paff3 = psB.tile([128, 2], dt.float32, tag="B", name="paff3")
            nc.tensor.matmul(paff1[:], selM[:, 0:128], sA[:], start=True, stop=True)
            nc.tensor.matmul(paff3[:], selM[:, 128:256], sA[:, 0:2],
                             start=True, stop=True)
            aff1 = spool.tile([128, 4], dt.float32, tag="aff1", name="aff1")
            aff3 = spool.tile([128, 2], dt.float32, tag="aff3", name="aff3")
            nc.scalar.copy(aff1[:], paff1[:])
            nc.scalar.copy(aff3[:], paff3[:])
            if taps:
                nc.sync.dma_start(d_taps["tap_aff1"].ap(), aff1[:])
                for cv in range(2):
                    for pk in range(PACKS):
                        nc.sync.dma_start(
                            d_taps[f"tap_a{cv + 1}T{pk}"].ap(), aT[cv][pk][:])
                nc.sync.dma_start(d_taps["tap_a3row"].ap(), a3row[:])

            pool4 = spool.tile([128, 4], dt.float32, tag="pool4", name="pool4")
            for cv in range(2):
                for pk in range(PACKS):
                    nc.scalar.activation(aT[cv][pk][:], aT[cv][pk][:],
                                         AF.Prelu, alpha=SLOPE,
                                         scale=aff1[:, 2 * cv:2 * cv + 1],
                                         bias=aff1[:, 2 * cv + 1:2 * cv + 2])
                    nc.vector.tensor_reduce(pool4[:, 2 * cv + pk:2 * cv + pk + 1],
                                            aT[cv][pk][:], axis=AX.X, op=OP.add)
            A3f = spool.tile([128, 24], dt.float32, tag="A3f", name="A3f")
            nc.scalar.activation(A3f[:], A3all[:], AF.Prelu, alpha=SLOPE,
                                 scale=aff3[:, 0:1], bias=aff3[:, 1:2])

            # ================================================================
            # Stage 5: assemble zT [332, 8], AllGather -> [332, 64]
            # ================================================================
            zT0 = spool.tile([64, GPC], dt.float32, tag="zT0", name="zT0")
            g_in = dpool.tile([332, GPC], dt.float32, name="g_in")
            for g in range(GPC):
                pk, q = g // 4, g % 4
                nc.sync.dma_start(zT0[0:32, g:g + 1],
                                  pool4[32 * q:32 * q + 32, pk:pk + 1])
                nc.sync.dma_start(zT0[32:64, g:g + 1],
                                  pool4[32 * q:32 * q + 32, 2 + pk:3 + pk])
                nc.sync.dma_start(g_in[64:192, g:g + 1],
                                  A3f[:, 3 * g:3 * g + 1])
                nc.sync.dma_start(g_in[192:320, g:g + 1],
                                  A3f[:, 3 * g + 1:3 * g + 2])
                nc.sync.dma_start(g_in[320:332, g:g + 1],
                                  A3f[0:12, 3 * g + 2:3 * g + 3])
            g_out = dpool.tile([NCORES, 332, GPC], dt.float32, name="g_out")
            nc.sync.dma_start(g_in[0:64, :], zT0[:])
            if single:
                for cc in range(NCORES):
                    nc.sync.dma_start(g_out[cc, :, :], g_in[:])
            else:
                nc.gpsimd.collective_compute(
                    "AllGather", OP.bypass, replica_groups=[list(range(NCORES))],
                    ins=[g_in[:]], outs=[g_out[:]])
            zAll = [spool.tile([sz, B], dt.float32, tag=f"zAll{i}", name=f"zAll{i}")
                    for i, (o, sz) in enumerate(MCH)]
            for i, (o, sz) in enumerate(MCH):
                nc.sync.dma_start(
                    zAll[i][:].rearrange("r (c j) -> r c j", j=GPC),
                    g_out[:, o:o + sz, :].transpose([1, 0, 2]))
            if taps:
                nc.sync.dma_start(d_taps["tap_za"].ap(), zAll[0][:])
                nc.sync.dma_start(d_taps["tap_zb"].ap(), zAll[1][:])
                nc.sync.dma_start(d_taps["tap_zc"].ap(), zAll[2][:])

            # ================================================================
            # Stage 6: readout MLP, replicated on every core
            # ================================================================
            def bn_affine_cols(pml, gbe_t, col0, tag):
                """pml psum [128, B]; returns (s, t) [128,1] tiles."""
                s_sum = spool.tile([128, 1], dt.float32, tag=f"{tag}_sum", name=f"{tag}_sum")
                q_sum = spool.tile([128, 1], dt.float32, tag=f"{tag}_q", name=f"{tag}_q")
                scr = spool.tile([128, B], dt.float32, tag="mlp_scr", name="mlp_scr")
                nc.vector.tensor_reduce(s_sum[:], pml[:], axis=AX.X, op=OP.add)
                nc.scalar.activation(scr[:], pml[:], AF.Square, accum_out=q_sum[:])
                nc.vector.tensor_scalar_mul(s_sum[:], s_sum[:], 1.0 / B)
                nc.vector.tensor_scalar_mul(q_sum[:], q_sum[:], 1.0 / B)
                v = spool.tile([128, 1], dt.float32, tag=f"{tag}_v", name=f"{tag}_v")
                nc.vector.tensor_tensor(v[:], s_sum[:], s_sum[:], op=OP.mult)
                nc.vector.tensor_tensor(v[:], q_sum[:], v[:], op=OP.subtract)
                nc.vector.tensor_scalar_add(v[:], v[:], EPS)
                nc.scalar.sqrt(v[:], v[:])
                nc.vector.reciprocal(v[:], v[:])
                s_t = spool.tile([128, 1], dt.float32, tag=f"{tag}_s", name=f"{tag}_s")
                t_t = spool.tile([128, 1], dt.float32, tag=f"{tag}_t", name=f"{tag}_t")
                nc.vector.tensor_tensor(s_t[:], gbe_t[:, col0:col0 + 1], v[:],
                                        op=OP.mult)
                nc.vector.tensor_tensor(t_t[:], s_sum[:], s_t[:], op=OP.mult)
                nc.vector.tensor_tensor(t_t[:], gbe_t[:, col0 + 1:col0 + 2],
                                        t_t[:], op=OP.subtract)
                return s_t, t_t

            z1T = [spool.tile([128, B], dt.float32, tag=f"z1T{m}", name=f"z1T{m}")
                   for m in range(2)]
            for m in range(2):
                ph1 = psM.tile([128, B], dt.float32, tag="M", name="ph1")
                for i, (o, sz) in enumerate(MCH):
                    nc.tensor.matmul(ph1[:], wl1[i][:, 128 * m:128 * m + 128],
                                     zAll[i][:], start=(i == 0), stop=(i == 2))
                s_t, t_t = bn_affine_cols(ph1[:], gbe4, 2 * m, f"bn4_{m}")
                nc.scalar.activation(z1T[m][:], ph1[:], AF.Prelu, alpha=SLOPE,
                                     scale=s_t[:], bias=t_t[:])
            ph2 = psM.tile([128, B], dt.float32, tag="M", name="ph2")
            for m in range(2):
                nc.tensor.matmul(ph2[:], wl2[m][:], z1T[m][:],
                                 start=(m == 0), stop=(m == 1))
            s_t, t_t = bn_affine_cols(ph2[:], gbe5, 0, "bn5")
            z2T = spool.tile([128, B], dt.float32, tag="z2T", name="z2T")
            nc.scalar.activation(z2T[:], ph2[:], AF.Prelu, alpha=SLOPE,
                                 scale=s_t[:], bias=t_t[:])
            pout = psM.tile([1, B], dt.float32, tag="M", name="pout")
            nc.tensor.matmul(pout[:], wl3[:], z2T[:], start=True, stop=True)
            out_sb = spool.tile([1, B], dt.float32, tag="out_sb", name="out_sb")
            nc.scalar.activation(out_sb[:], pout[:], AF.Identity, bias=bl3[:])
            nc.sync.dma_start(d_out.ap(), out_sb[:])

    nc.compile()
    return nc


def _host_prep(inputs):
    x = np.asarray(inputs["x"])
    ei = np.asarray(inputs["edge_index"])
    src = ei[0]
    tgt = ei[1]
    exp_tgt = np.repeat(np.arange(NTOT, dtype=np.int64), DEG)
    assert np.array_equal(tgt.astype(np.int64), exp_tgt), \
        "edge_index structure mismatch (expected DGCN static grid)"
    assert np.array_equal(src // ROI, tgt // ROI), "cross-graph edges found"
    src_local = (src % ROI).astype(np.int16).reshape(B, ROI, DEG)

    # wrapped edge lists: el [B, EG]; wr [B, 16, EG//16]
    el = src_local.reshape(B, EG)
    wr = el.reshape(B, EG // 16, 16).transpose(0, 2, 1)  # pos n=(col*16+p)

    W1a, W2a, W3 = (_fp32(inputs[k]) for k in ("W1a", "W2a", "W3"))
    A1, B1 = W1a[:F] - W1a[F:], W1a[F:]
    A2, B2 = W2a[:F] - W2a[F:], W2a[F:]
    A3, B3 = W3[:F] - W3[F:], W3[F:]

    wproj = _fp32(np.concatenate([A1, B1, A2, B2], axis=1))
    wproj3 = _fp32(np.concatenate([A3, B3], axis=1))

    import ml_dtypes

    def blockdiag(w):
        out = np.zeros((128, 128), np.float32)
        for q in range(4):
            out[32 * q:32 * q + 32, 32 * q:32 * q + 32] = w
        return out

    wbd1 = blockdiag(_fp32(inputs["W1b"])).astype(ml_dtypes.bfloat16)
    wbd2 = blockdiag(_fp32(inputs["W2b"])).astype(ml_dtypes.bfloat16)
    bpack = _fp32(np.stack([np.tile(inputs["b1a"], 4),
                            np.tile(inputs["b2a"], 4)], axis=1))
    ident = np.eye(128, dtype=np.float32)

    foldM = np.zeros((128, 33), np.float32)
    p = np.arange(128)
    foldM[p, p % 32] = 1.0
    foldM[:, 32] = 1.0
    sel32 = np.zeros((33, 128), np.float32)
    sel32[p % 32, p] = 1.0
    sel3 = np.zeros((33, 128), np.float32)
    sel3[32, :] = 1.0
    selM = _fp32(np.concatenate([sel32, sel3], axis=1))

    gbe = np.zeros((33, 4), np.float32)
    gbe[0:32, 0] = inputs["g1"]
    gbe[0:32, 1] = inputs["be1"]
    gbe[0:32, 2] = inputs["g2"]
    gbe[0:32, 3] = inputs["be2"]
    gbe[32, 0] = inputs["g3"][0]
    gbe[32, 1] = inputs["be3"][0]


    wl1 = _fp32(inputs["Wl1"]).copy()
    wl1[0:64, :] /= ROI          # mean pooling folded into the weights
    wl2 = _fp32(inputs["Wl2"])
    wl3 = _fp32(inputs["Wl3"])
    gbe4 = _fp32(np.stack([inputs["g4"][0:128], inputs["be4"][0:128],
                           inputs["g4"][128:256], inputs["be4"][128:256]],
                          axis=1))
    gbe5 = _fp32(np.stack([inputs["g5"], inputs["be5"]], axis=1))
    bl3 = _fp32(inputs["bl3"].reshape(1, 1))

    shared = dict(ident=ident, wproj=wproj, wproj3=wproj3, wbd1=wbd1,
                  wbd2=wbd2, bpack=bpack, foldM=foldM, selM=selM, gbe=gbe,
                  wl1=wl1, wl2=wl2, wl3=wl3, gbe4=gbe4,
                  gbe5=gbe5, bl3=bl3)

    in_maps = []
    for c in range(NCORES):
        gs = slice(GPC * c, GPC * (c + 1))
        xT = _fp32(x[NLOC * c:NLOC * (c + 1)].T)
        idx1 = np.empty((2 * 128, EG // 16), np.int16)
        for t in range(2):
            for qq in range(4):
                g = GPC * c + 4 * t + qq
                idx1[128 * t + 32 * qq:128 * t + 32 * qq + 16] = wr[g]
                idx1[128 * t + 32 * qq + 16:128 * t + 32 * qq + 32] = wr[g]
        mask3 = np.full((128, 24, ROI), -1e30, np.float32)
        for g in range(GPC):
            sl = el[GPC * c + g].reshape(ROI, DEG)
            for cc3, csz3 in enumerate((128, 128, 12)):
                for pp in range(csz3):
                    mask3[pp, 3 * g + cc3, sl[128 * cc3 + pp]] = 0.0
        mask3 = mask3.reshape(128, 24 * ROI).astype(ml_dtypes.bfloat16)
        m = dict(shared)
        m.update(xT=xT, idx1=np.ascontiguousarray(idx1),
                 mask3=np.ascontiguousarray(mask3))
        in_maps.append(m)
    return in_maps


def kernel(**inputs):
    from concourse.bass_utils import run_bass_kernel_spmd

    if "nc" not in _cache:
        _cache["nc"] = _build_program()
    nc = _cache["nc"]
    in_maps = _host_prep(inputs)
    trace = bool(int(os.environ.get("KERNEL_TRACE", "0")))
    tmpdir = os.environ.get("KERNEL_TMPDIR") or None
    if tmpdir:
        os.makedirs(tmpdir, exist_ok=True)
    res = run_bass_kernel_spmd(nc, in_maps, core_ids=list(range(NCORES)),
                               trace=trace, tmpdir=tmpdir)
    _cache["last_results"] = res
    out = res.results[0]["out"].reshape(B, 1)
    return np.ascontiguousarray(out, dtype=np.float32)

